# revision 1
# baseline (speedup 1.0000x reference)
"""Trainium2 Bass kernel for a pre-LN transformer block (dense_transformer).

Problem shapes (hardcoded): x [32, 577, 768], 12 heads, dh=64, mlp 3072.
NOTE: softmax in the reference is over the HEADS axis (dim=1 of [B,h,T,T]),
replicated faithfully here.

Sharding: pure data-parallel over batch: 8 cores x 4 batches each.
Weights replicated. No collectives.

Layout strategy inside each core: activations kept feature-major
([feature on partitions, tokens on free dim]) so every matmul consumes
natural weight tiles [k,n] and produces feature-major outputs directly.
Only the kernel input/output get PE-transposed between token-major and
feature-major. Matmuls run in bf16 with fp32 PSUM accumulation.
"""
import os as _os

import numpy as np

import concourse.bacc as bacc
import concourse.mybir as mybir
import concourse.tile as tile
from concourse.bass_utils import run_bass_kernel_spmd
from concourse.masks import make_identity

F32 = mybir.dt.float32
BF16 = mybir.dt.bfloat16
AF = mybir.ActivationFunctionType
ALU = mybir.AluOpType

N_CORES = 8
B, T, D = 32, 577, 768
BL = B // N_CORES          # 4 batches per core
NH, DH = 12, 64            # heads
HID = 4 * D                # 3072
KD = D // 128              # 6 feature tiles
KH = HID // 128            # 24 hidden tiles
EPS = 1e-6
SCALE = DH ** -0.5
GELU_FUNC = None           # sim_test overrides with Tanh (CoreSim lacks Gelu)
KB_PHASE = _os.environ.get("KB_PHASE", "ab")   # "a1", "a", "ab" (bisection)

TOK = BL * T               # 2308 tokens per core
CH_T = [(0, 512), (512, 65)]                      # within one batch (577)
CH_Q = [(0, 289), (289, 288)]                     # attention qt chunks
CH_K = [(0, 128), (128, 128), (256, 128), (384, 128), (512, 65)]  # kt tiles
CH_G = [(0, 512), (512, 512), (1024, 512), (1536, 512), (2048, 260)]  # global

_NC_CACHE = {}


def _chunks(total, step):
    out = []
    o = 0
    while o < total:
        out.append((o, min(step, total - o)))
        o += step
    return out


def build_nc():
    nc = bacc.Bacc("TRN2")
    x = nc.dram_tensor("x", [BL, T, D], F32, kind="ExternalInput")
    ln1_w = nc.dram_tensor("ln1_w", [D], F32, kind="ExternalInput")
    ln1_b = nc.dram_tensor("ln1_b", [D], F32, kind="ExternalInput")
    qkv_w = nc.dram_tensor("qkv_w", [D, 3 * D], F32, kind="ExternalInput")
    qkv_b = nc.dram_tensor("qkv_b", [3 * D], F32, kind="ExternalInput")
    proj_w = nc.dram_tensor("proj_w", [D, D], F32, kind="ExternalInput")
    proj_b = nc.dram_tensor("proj_b", [D], F32, kind="ExternalInput")
    ln2_w = nc.dram_tensor("ln2_w", [D], F32, kind="ExternalInput")
    ln2_b = nc.dram_tensor("ln2_b", [D], F32, kind="ExternalInput")
    fc1_w = nc.dram_tensor("fc1_w", [D, HID], F32, kind="ExternalInput")
    fc1_b = nc.dram_tensor("fc1_b", [HID], F32, kind="ExternalInput")
    fc2_w = nc.dram_tensor("fc2_w", [HID, D], F32, kind="ExternalInput")
    fc2_b = nc.dram_tensor("fc2_b", [D], F32, kind="ExternalInput")
    out = nc.dram_tensor("out", [BL, T, D], F32, kind="ExternalOutput")

    # DRAM scratch
    x2_dram = nc.dram_tensor("x2_dram", [KD, 128, TOK], F32, kind="Internal")

    x_flat = x.ap().rearrange("b t d -> (b t) d")      # [2308, 768]
    out_flat = out.ap().rearrange("b t d -> (b t) d")

    with tile.TileContext(nc) as tc:
        with tc.tile_pool(name="persist", bufs=1) as pp:
            # ---- constants / biases ----
            ident = pp.tile([128, 128], F32)
            make_identity(nc, ident)
            ones_f = pp.tile([128, 1], F32)
            nc.vector.memset(ones_f, 1.0)
            ones_b = pp.tile([128, 1], BF16)
            nc.vector.memset(ones_b, 1.0)
            ones_r = pp.tile([1, 128], F32)   # broadcast lhsT (partition 0)
            nc.vector.memset(ones_r, 1.0)
            eps_t = pp.tile([1, 1], F32)
            nc.vector.memset(eps_t, EPS)

            def load_cols(name, dram_ap, n):
                t = pp.tile([128, n], F32, tag=name)
                nc.sync.dma_start(
                    out=t, in_=dram_ap.rearrange("(n p) -> p n", p=128))
                return t

            qkvb_sb = load_cols("qkvb", qkv_b.ap(), 18)
            projb_sb = load_cols("projb", proj_b.ap(), 6)
            fc1b_sb = load_cols("fc1b", fc1_b.ap(), 24)
            fc2b_sb = load_cols("fc2b", fc2_b.ap(), 6)
            ln1w_sb = load_cols("ln1w", ln1_w.ap(), 6)
            ln1b_sb = load_cols("ln1b", ln1_b.ap(), 6)
            ln2w_sb = load_cols("ln2w", ln2_w.ap(), 6)
            ln2b_sb = load_cols("ln2b", ln2_b.ap(), 6)
            # v-bias broadcast along partitions [128, 768]
            vb_bc = pp.tile([128, D], F32)
            nc.sync.dma_start(
                out=vb_bc,
                in_=qkv_b.ap()[2 * D:3 * D]
                .rearrange("(one d) -> one d", one=1).to_broadcast((128, D)))

            def layernorm(srcb, w_sb, b_sb, dst3, chunks, psum_pool, rot_pool):
                """srcb: [128, KD, T*] feature-major BF16 copy of the input;
                dst3: bf16 output. Stats via ones-matmul on PE; broadcast via
                PE; apply fully in bf16 (DVE 2x/4x modes)."""
                ncols = sum(c[1] for c in chunks)
                mu = rot_pool.tile([1, ncols], F32, tag="mu", bufs=1)
                msq = rot_pool.tile([1, ncols], F32, tag="msq", bufs=1)
                for t0, cnt in chunks:
                    ps_s = psum_pool.tile([1, 512], F32, tag="rot")
                    for k in range(KD):
                        nc.tensor.matmul(ps_s[:, :cnt], ones_b,
                                         srcb[:, k, t0:t0 + cnt],
                                         start=(k == 0), stop=(k == KD - 1))
                    nc.scalar.mul(out=mu[:, t0:t0 + cnt], in_=ps_s[:, :cnt],
                                  mul=1.0 / D)
                for t0, cnt in chunks:
                    ps_q = psum_pool.tile([1, 512], F32, tag="rot")
                    for k in range(KD):
                        sq = rot_pool.tile([128, 577], BF16, tag="sq")
                        nc.vector.tensor_mul(out=sq[:, :cnt],
                                             in0=srcb[:, k, t0:t0 + cnt],
                                             in1=srcb[:, k, t0:t0 + cnt])
                        nc.tensor.matmul(ps_q[:, :cnt], ones_b, sq[:, :cnt],
                                         start=(k == 0), stop=(k == KD - 1))
                    nc.scalar.mul(out=msq[:, t0:t0 + cnt], in_=ps_q[:, :cnt],
                                  mul=1.0 / D)
                var = rot_pool.tile([1, ncols], F32, tag="var", bufs=1)
                nc.vector.tensor_mul(out=var, in0=mu, in1=mu)
                nc.vector.tensor_sub(out=var, in0=msq, in1=var)
                nc.scalar.activation(out=var, in_=var, func=AF.Sqrt, bias=eps_t)
                rstd = rot_pool.tile([1, ncols], F32, tag="rstd", bufs=1)
                nc.vector.reciprocal(out=rstd, in_=var)
                # broadcast mu & rstd across partitions via PE -> sbuf bf16
                for t0, cnt in chunks:
                    ps_mu = psum_pool.tile([128, 512], F32, tag="rot")
                    nc.tensor.matmul(ps_mu[:, :cnt], ones_r, mu[:, t0:t0 + cnt],
                                     start=True, stop=True)
                    ps_rs = psum_pool.tile([128, 512], F32, tag="rot")
                    nc.tensor.matmul(ps_rs[:, :cnt], ones_r, rstd[:, t0:t0 + cnt],
                                     start=True, stop=True)
                    mu_b = rot_pool.tile([128, 577], BF16, tag="mu_b")
                    nc.vector.tensor_copy(out=mu_b[:, :cnt], in_=ps_mu[:, :cnt])
                    rs_b = rot_pool.tile([128, 577], BF16, tag="rs_b")
                    nc.vector.tensor_copy(out=rs_b[:, :cnt], in_=ps_rs[:, :cnt])
                    for k in range(KD):
                        t1 = rot_pool.tile([128, 577], BF16, tag="lnt1")
                        nc.vector.tensor_sub(out=t1[:, :cnt],
                                             in0=srcb[:, k, t0:t0 + cnt],
                                             in1=mu_b[:, :cnt])
                        t2 = rot_pool.tile([128, 577], BF16, tag="lnt2")
                        nc.vector.tensor_mul(out=t2[:, :cnt], in0=t1[:, :cnt],
                                             in1=rs_b[:, :cnt])
                        nc.vector.tensor_scalar(
                            out=dst3[:, k, t0:t0 + cnt], in0=t2[:, :cnt],
                            scalar1=w_sb[:, k:k + 1], scalar2=b_sb[:, k:k + 1],
                            op0=ALU.mult, op1=ALU.add)

            # ============ PHASE A ============
            with tc.tile_pool(name="pa", bufs=1) as pa, \
                 tc.tile_pool(name="stg", bufs=2) as stg, \
                 tc.tile_pool(name="rot2", bufs=2) as rot2, \
                 tc.tile_pool(name="ps_rot", bufs=3, space="PSUM") as ps_rot, \
                 tc.tile_pool(name="ps_wa", bufs=3, space="PSUM") as ps_wa, \
                 tc.tile_pool(name="ps_mm", bufs=2, space="PSUM") as ps_mm:

                # resident bf16 weights for qkv & proj (cast from fp32)
                qkvw = pa.tile([128, KD, 3 * D], BF16, tag="qkvw")
                projw = pa.tile([128, KD, D], BF16, tag="projw")

                def cast_weights(dram, ktiles, ncols, dst=None, dst_dram=None,
                                 eng_flip=0):
                    i = 0
                    for k in range(ktiles):
                        for c0, cw in _chunks(ncols, 512):
                            st = stg.tile([128, 512], F32, tag="wstage")
                            nc.sync.dma_start(
                                out=st[:, :cw],
                                in_=dram[k * 128:(k + 1) * 128, c0:c0 + cw])
                            if dst is not None:
                                dd = dst[:, k, c0:c0 + cw]
                                if (i + eng_flip) % 2 == 0:
                                    nc.vector.tensor_copy(out=dd, in_=st[:, :cw])
                                else:
                                    nc.scalar.copy(out=dd, in_=st[:, :cw])
                            else:
                                bt = stg.tile([128, 512], BF16, tag="bstage")
                                if (i + eng_flip) % 2 == 0:
                                    nc.vector.tensor_copy(out=bt[:, :cw], in_=st[:, :cw])
                                else:
                                    nc.scalar.copy(out=bt[:, :cw], in_=st[:, :cw])
                                nc.sync.dma_start(
                                    out=dst_dram.ap()[k, :, c0:c0 + cw],
                                    in_=bt[:, :cw])
                            i += 1

                cast_weights(qkv_w.ap(), KD, 3 * D, dst=qkvw)
                cast_weights(proj_w.ap(), KD, D, dst=projw, eng_flip=1)

                n_batches = 1 if KB_PHASE == "a1" else BL
                for b in range(n_batches):
                    # ---- load + transpose x into feature-major fp32 ----
                    xT = pa.tile([128, KD, T], F32, tag="xT")
                    for tt, (t0, cnt) in enumerate(CH_K):
                        tmx = stg.tile([128, D], F32, tag="xstage")
                        nc.sync.dma_start(
                            out=tmx[:cnt, :],
                            in_=x_flat[b * T + t0: b * T + t0 + cnt, :])
                        for k in range(KD):
                            ps_tr = ps_mm.tile([128, 128], F32, tag="mm")
                            nc.tensor.transpose(ps_tr[:, :cnt],
                                                tmx[:cnt, k * 128:(k + 1) * 128],
                                                ident[:cnt, :cnt])
                            nc.scalar.copy(out=xT[:, k, t0:t0 + cnt],
                                           in_=ps_tr[:, :cnt])

                    # ---- LN1 -> yT bf16 ----
                    xTb = pa.tile([128, KD, T], BF16, tag="xTb")
                    for k in range(KD):
                        nc.vector.tensor_copy(out=xTb[:, k, :], in_=xT[:, k, :])
                    yT = pa.tile([128, KD, T], BF16, tag="yT")
                    layernorm(xTb, ln1w_sb, ln1b_sb, yT, CH_T, ps_rot, rot2)

                    # ---- q,k (feature-major) ----
                    qkT = pa.tile([128, NH, T], BF16, tag="qkT")
                    for n in range(NH):
                        for t0, cnt in CH_T:
                            ps = ps_mm.tile([128, 512], F32, tag="mm")
                            for k in range(KD):
                                nc.tensor.matmul(ps[:, :cnt],
                                                 qkvw[:, k, n * 128:(n + 1) * 128],
                                                 yT[:, k, t0:t0 + cnt],
                                                 start=(k == 0), stop=(k == KD - 1))
                            nc.scalar.activation(out=qkT[:, n, t0:t0 + cnt],
                                                 in_=ps[:, :cnt],
                                                 func=AF.Identity,
                                                 bias=qkvb_sb[:, n:n + 1])

                    # ---- v (token-major) ----
                    v_sb = pa.tile([128, len(CH_K), D], BF16, tag="v")
                    for tt, (t0, cnt) in enumerate(CH_K):
                        for f0, fw in ((0, 384), (384, 384)):
                            ps = ps_mm.tile([128, 512], F32, tag="mm")
                            for k in range(KD):
                                nc.tensor.matmul(ps[:cnt, :fw],
                                                 yT[:, k, t0:t0 + cnt],
                                                 qkvw[:, k, 2 * D + f0:2 * D + f0 + fw],
                                                 start=(k == 0), stop=(k == KD - 1))
                            nc.vector.tensor_add(out=v_sb[:cnt, tt, f0:f0 + fw],
                                                 in0=ps[:cnt, :fw],
                                                 in1=vb_bc[:cnt, f0:f0 + fw])

                    # ---- attention (softmax over heads!) ----
                    # wa accumulates over kt directly in PSUM; head pairs are
                    # col-packed into one bank via tile_position (partition-
                    # scoped has_written -> interleaved groups are safe).
                    # Heads 0-5 accumulate during the main kt loop; heads 6-11
                    # replay from stored attn weights in a second PE-dense
                    # pass using the same 3 banks.
                    waB = pa.tile([128, KD, T], BF16, tag="waB")
                    for q0, qcnt in CH_Q:
                        wapA = [ps_wa.tile([128, 289], F32, tag="wa",
                                            name=f"wapA{_j}")
                                for _j in range(3)]
                        attn_p = pa.tile([128, len(CH_K), 6, 289], BF16,
                                         tag="attnp")
                        for kt, (k0, kcnt) in enumerate(CH_K):
                            e_t = rot2.tile([128, NH, 289], BF16, tag="e", bufs=2)
                            for h in range(NH):
                                pb = (h % 2) * 64
                                ps_dp = ps_rot.tile([128, 289], F32, tag="rot")
                                nc.tensor.matmul(
                                    ps_dp[:kcnt, :qcnt],
                                    qkT[pb:pb + 64, 6 + h // 2, k0:k0 + kcnt],
                                    qkT[pb:pb + 64, h // 2, q0:q0 + qcnt],
                                    start=True, stop=True, tile_position=(pb, 0))
                                nc.scalar.activation(out=e_t[:kcnt, h, :qcnt],
                                                     in_=ps_dp[:kcnt, :qcnt],
                                                     func=AF.Exp, scale=SCALE)
                            zp = rot2.tile([128, 6, 289], BF16, tag="zp",
                                           bufs=2)
                            for j in range(6):
                                nc.vector.tensor_add(out=zp[:kcnt, j, :qcnt],
                                                     in0=e_t[:kcnt, 2 * j, :qcnt],
                                                     in1=e_t[:kcnt, 2 * j + 1, :qcnt])
                            z4a = rot2.tile([128, 289], BF16, tag="z4a", bufs=2)
                            nc.vector.tensor_add(out=z4a[:kcnt, :qcnt],
                                                 in0=zp[:kcnt, 0, :qcnt],
                                                 in1=zp[:kcnt, 1, :qcnt])
                            z4b = rot2.tile([128, 289], BF16, tag="z4b", bufs=2)
                            nc.vector.tensor_add(out=z4b[:kcnt, :qcnt],
                                                 in0=zp[:kcnt, 2, :qcnt],
                                                 in1=zp[:kcnt, 3, :qcnt])
                            z4c = rot2.tile([128, 289], BF16, tag="z4c", bufs=2)
                            nc.vector.tensor_add(out=z4c[:kcnt, :qcnt],
                                                 in0=zp[:kcnt, 4, :qcnt],
                                                 in1=zp[:kcnt, 5, :qcnt])
                            z8 = rot2.tile([128, 289], BF16, tag="z8", bufs=2)
                            nc.vector.tensor_add(out=z8[:kcnt, :qcnt],
                                                 in0=z4a[:kcnt, :qcnt],
                                                 in1=z4b[:kcnt, :qcnt])
                            z = rot2.tile([128, 289], F32, tag="z", bufs=2)
                            nc.vector.tensor_add(out=z[:kcnt, :qcnt],
                                                 in0=z8[:kcnt, :qcnt],
                                                 in1=z4c[:kcnt, :qcnt])
                            rz = rot2.tile([128, 289], F32, tag="rz", bufs=2)
                            nc.vector.reciprocal(out=rz[:kcnt, :qcnt],
                                                 in_=z[:kcnt, :qcnt])
                            rzb = rot2.tile([128, 289], BF16, tag="rzb", bufs=2)
                            nc.vector.tensor_copy(out=rzb[:kcnt, :qcnt],
                                                  in_=rz[:kcnt, :qcnt])
                            attn = rot2.tile([128, 6, 289], BF16, tag="attn", bufs=2)
                            for h in range(6):
                                nc.vector.tensor_mul(out=attn[:kcnt, h, :qcnt],
                                                     in0=e_t[:kcnt, h, :qcnt],
                                                     in1=rzb[:kcnt, :qcnt])
                            for h in range(6, NH):
                                nc.vector.tensor_mul(
                                    out=attn_p[:kcnt, kt, h - 6, :qcnt],
                                    in0=e_t[:kcnt, h, :qcnt],
                                    in1=rzb[:kcnt, :qcnt])
                            for h in range(6):
                                pb = (h % 2) * 64
                                nc.tensor.matmul(
                                    wapA[h // 2][pb:pb + 64, :qcnt],
                                    v_sb[:kcnt, kt, h * 64:(h + 1) * 64],
                                    attn[:kcnt, h, :qcnt],
                                    start=(kt == 0), stop=(kt == len(CH_K) - 1),
                                    tile_position=(0, pb),
                                    skip_group_check=True)
                        for j in range(3):
                            nc.scalar.copy(out=waB[:, j, q0:q0 + qcnt],
                                           in_=wapA[j][:, :qcnt])
                        wapB = [ps_wa.tile([128, 289], F32, tag="wa",
                                            name=f"wapB{_j}")
                                for _j in range(3)]
                        for kt, (k0, kcnt) in enumerate(CH_K):
                            for h in range(6, NH):
                                pb = (h % 2) * 64
                                nc.tensor.matmul(
                                    wapB[(h - 6) // 2][pb:pb + 64, :qcnt],
                                    v_sb[:kcnt, kt, h * 64:(h + 1) * 64],
                                    attn_p[:kcnt, kt, h - 6, :qcnt],
                                    start=(kt == 0), stop=(kt == len(CH_K) - 1),
                                    tile_position=(0, pb),
                                    skip_group_check=True)
                        for j in range(3):
                            nc.scalar.copy(out=waB[:, 3 + j, q0:q0 + qcnt],
                                           in_=wapB[j][:, :qcnt])

                    # ---- proj + residual -> xT2; stash to DRAM ----
                    xT2 = pa.tile([128, KD, T], F32, tag="xT2")
                    for n in range(KD):
                        for t0, cnt in CH_T:
                            ps = ps_mm.tile([128, 512], F32, tag="mm")
                            for k in range(KD):
                                nc.tensor.matmul(ps[:, :cnt],
                                                 projw[:, k, n * 128:(n + 1) * 128],
                                                 waB[:, k, t0:t0 + cnt],
                                                 start=(k == 0), stop=(k == KD - 1))
                            nc.vector.scalar_tensor_tensor(
                                out=xT2[:, n, t0:t0 + cnt], in0=ps[:, :cnt],
                                scalar=projb_sb[:, n:n + 1],
                                in1=xT[:, n, t0:t0 + cnt],
                                op0=ALU.add, op1=ALU.add)
                    for k in range(KD):
                        nc.sync.dma_start(out=x2_dram.ap()[k, :, b * T:(b + 1) * T],
                                            in_=xT2[:, k, :])


            # ============ PHASE B (MLP over global tokens) ============
            if KB_PHASE != "ab":
                with tc.tile_pool(name="pbz", bufs=1) as pbz:
                    zt = pbz.tile([128, D], F32)
                    nc.vector.memset(zt, 0.0)
                    for g0, cnt in CH_G:
                        for c0, ccnt in _chunks(cnt, 128):
                            nc.sync.dma_start(
                                out=out_flat[g0 + c0:g0 + c0 + ccnt, :],
                                in_=zt[:ccnt, :])
            else:
                with tc.tile_pool(name="pb", bufs=1) as pb, \
                     tc.tile_pool(name="pb2", bufs=2) as pb2, \
                     tc.tile_pool(name="pb3", bufs=3) as pb3, \
                     tc.tile_pool(name="ps_rotB", bufs=3, space="PSUM") as ps_rotB, \
                     tc.tile_pool(name="ps_mmB", bufs=3, space="PSUM") as ps_mmB:

                    fc1w = pb.tile([128, KD, HID], BF16, tag="fc1w")
                    fc2w = pb.tile([128, KH, D], BF16, tag="fc2w")

                    def cast_in(dram, ktiles, ncols, dst, eng_flip=0):
                        i = 0
                        for k in range(ktiles):
                            for c0, cw in _chunks(ncols, 512):
                                st = pb3.tile([128, 512], F32, tag="wstgB",
                                              bufs=4)
                                nc.sync.dma_start(
                                    out=st[:, :cw],
                                    in_=dram[k * 128:(k + 1) * 128, c0:c0 + cw])
                                dd = dst[:, k, c0:c0 + cw]
                                if (i + eng_flip) % 2 == 0:
                                    nc.vector.tensor_copy(out=dd, in_=st[:, :cw])
                                else:
                                    nc.scalar.copy(out=dd, in_=st[:, :cw])
                                i += 1

                    cast_in(fc1_w.ap(), KD, HID, fc1w)
                    cast_in(fc2_w.ap(), KH, D, fc2w, eng_flip=1)

                    for g0, cnt in CH_G:
                        xc = pb2.tile([128, KD, 512], F32, tag="xc")
                        for k in range(KD):
                            nc.sync.dma_start(out=xc[:, k, :cnt],
                                                in_=x2_dram.ap()[k, :, g0:g0 + cnt])
                        xcb = pb2.tile([128, KD, 512], BF16, tag="xcb", bufs=1)
                        for k in range(KD):
                            nc.vector.tensor_copy(out=xcb[:, k, :cnt],
                                                  in_=xc[:, k, :cnt])
                        zT = pb2.tile([128, KD, 512], BF16, tag="zT", bufs=1)
                        layernorm(xcb, ln2w_sb, ln2b_sb, zT, [(0, cnt)],
                                  ps_rotB, pb2)
                        h_t = pb.tile([128, KH, 512], BF16, tag="h")
                        for n in range(KH):
                            ps = ps_mmB.tile([128, 512], F32, tag="mmB")
                            for k in range(KD):
                                nc.tensor.matmul(ps[:, :cnt],
                                                 fc1w[:, k, n * 128:(n + 1) * 128],
                                                 zT[:, k, :cnt],
                                                 start=(k == 0), stop=(k == KD - 1))
                            nc.scalar.activation(out=h_t[:, n, :cnt], in_=ps[:, :cnt],
                                                 func=GELU_FUNC or AF.Gelu,
                                                 bias=fc1b_sb[:, n:n + 1])
                        xf = pb2.tile([128, KD, 512], F32, tag="xf", bufs=1)
                        for n in range(KD):
                            ps = ps_mmB.tile([128, 512], F32, tag="mmB")
                            for k in range(KH):
                                nc.tensor.matmul(ps[:, :cnt],
                                                 fc2w[:, k, n * 128:(n + 1) * 128],
                                                 h_t[:, k, :cnt],
                                                 start=(k == 0), stop=(k == KH - 1))
                            nc.vector.scalar_tensor_tensor(
                                out=xf[:, n, :cnt], in0=ps[:, :cnt],
                                scalar=fc2b_sb[:, n:n + 1], in1=xc[:, n, :cnt],
                                op0=ALU.add, op1=ALU.add)
                        # ---- transpose back to token-major and store ----
                        for c0, ccnt in _chunks(cnt, 128):
                            om = pb3.tile([128, D], F32, tag="om", bufs=2)
                            for k in range(KD):
                                ps_tr = ps_rotB.tile([128, 128], F32, tag="rot")
                                nc.tensor.transpose(ps_tr[:ccnt, :],
                                                    xf[:, k, c0:c0 + ccnt], ident)
                                nc.scalar.copy(out=om[:ccnt, k * 128:(k + 1) * 128],
                                               in_=ps_tr[:ccnt, :])
                            nc.sync.dma_start(
                                out=out_flat[g0 + c0:g0 + c0 + ccnt, :],
                                in_=om[:ccnt, :])

    nc.compile()
    return nc


def kernel(**inputs) -> np.ndarray:
    if "nc" in _NC_CACHE:
        nc = _NC_CACHE["nc"]
    else:
        nc = _NC_CACHE["nc"] = build_nc()
    x = np.ascontiguousarray(np.asarray(inputs["x"], dtype=np.float32))
    weights = {k: np.ascontiguousarray(np.asarray(v, dtype=np.float32))
               for k, v in inputs.items() if k != "x"}
    in_maps = []
    for c in range(N_CORES):
        m = {"x": x[c * BL:(c + 1) * BL]}
        m.update(weights)
        in_maps.append(m)
    last_err = None
    for attempt in range(3):
        try:
            r = run_bass_kernel_spmd(nc, in_maps, core_ids=list(range(N_CORES)))
            return np.concatenate([r.results[c]["out"] for c in range(N_CORES)],
                                  axis=0)
        except Exception as e:  # transient device flakes: retry
            last_err = e
    raise last_err



# revision 10
# speedup vs baseline: 62.6261x; 62.6261x over previous
"""Trainium2 Bass kernel for a pre-LN transformer block (dense_transformer).

Problem shapes (hardcoded): x [32, 577, 768], 12 heads, dh=64, mlp 3072.
NOTE: softmax in the reference is over the HEADS axis (dim=1 of [B,h,T,T]),
replicated faithfully here.

Sharding: pure data-parallel over batch: 8 cores x 4 batches each.
Weights replicated. No collectives.

Layout: activations feature-major ([feature partitions, tokens free]) so
matmuls consume natural weight tiles [k,n]; matmuls in bf16 with fp32 PSUM.

Structure for PE density (HAM warmth): phase A emits batch b+1's front
(load/LN1/qkv/v) before batch b's attention so the scheduler backfills the
PE during softmax stretches; proj runs per q-chunk. Phase B (MLP) uses
LN2 stats precomputed in phase A and double-buffered token chunks.

Softmax-over-heads: paired-PSUM-bank exp, batched multi-dim DVE tree-sum,
reciprocal_approx_fast, head-broadcast normalize.

LN scale/shift are folded into the following matmul: W' = diag(w) @ W at
weight-cast time and b' = b_ln @ W + b at kernel start (tiny PE matmuls +
a DRAM roundtrip to relayout the folded bias row per-partition).
"""
import numpy as np

import concourse.bacc as bacc
import concourse.mybir as mybir
import concourse.tile as tile
from concourse.bass import broadcast_tensor_aps
from concourse.bass_utils import run_bass_kernel_spmd
from concourse.masks import make_identity

F32 = mybir.dt.float32
BF16 = mybir.dt.bfloat16
AF = mybir.ActivationFunctionType
ALU = mybir.AluOpType

N_CORES = 8
B, T, D = 32, 577, 768
BL = B // N_CORES          # 4 batches per core
NH, DH = 12, 64            # heads
HID = 4 * D                # 3072
KD = D // 128              # 6 feature tiles
KH = HID // 128            # 24 hidden tiles
EPS = 1e-6
SCALE = DH ** -0.5

TOK = BL * T               # 2308 tokens per core
CH_T = [(0, 512), (512, 65)]                      # within one batch (577)
CH_Q = [(0, 289), (289, 288)]                     # attention q chunks
CH_K = [(0, 128), (128, 128), (256, 128), (384, 128), (512, 65)]  # kt tiles
CH_G = [(0, 512), (512, 512), (1024, 512), (1536, 512), (2048, 260)]  # global

_NC_CACHE = {}


def _chunks(total, step):
    out = []
    o = 0
    while o < total:
        out.append((o, min(step, total - o)))
        o += step
    return out


def build_nc():
    nc = bacc.Bacc("TRN2")
    x = nc.dram_tensor("x", [BL, T, D], F32, kind="ExternalInput")
    ln1_w = nc.dram_tensor("ln1_w", [D], F32, kind="ExternalInput")
    ln1_b = nc.dram_tensor("ln1_b", [D], F32, kind="ExternalInput")
    qkv_w = nc.dram_tensor("qkv_w", [D, 3 * D], F32, kind="ExternalInput")
    qkv_b = nc.dram_tensor("qkv_b", [3 * D], F32, kind="ExternalInput")
    proj_w = nc.dram_tensor("proj_w", [D, D], F32, kind="ExternalInput")
    proj_b = nc.dram_tensor("proj_b", [D], F32, kind="ExternalInput")
    ln2_w = nc.dram_tensor("ln2_w", [D], F32, kind="ExternalInput")
    ln2_b = nc.dram_tensor("ln2_b", [D], F32, kind="ExternalInput")
    fc1_w = nc.dram_tensor("fc1_w", [D, HID], F32, kind="ExternalInput")
    fc1_b = nc.dram_tensor("fc1_b", [HID], F32, kind="ExternalInput")
    fc2_w = nc.dram_tensor("fc2_w", [HID, D], F32, kind="ExternalInput")
    fc2_b = nc.dram_tensor("fc2_b", [D], F32, kind="ExternalInput")
    out = nc.dram_tensor("out", [BL, T, D], F32, kind="ExternalOutput")

    # DRAM scratch
    x2_dram = nc.dram_tensor("x2_dram", [KD, 128, TOK], BF16, kind="Internal")
    qb2_dram = nc.dram_tensor("qb2_dram", [3 * D], BF16, kind="Internal")
    fb2_dram = nc.dram_tensor("fb2_dram", [HID], BF16, kind="Internal")

    x_flat = x.ap().rearrange("b t d -> (b t) d")      # [2308, 768]
    out_flat = out.ap().rearrange("b t d -> (b t) d")

    with tile.TileContext(nc) as tc:
        with tc.tile_pool(name="persist", bufs=1) as pp:
            ident = pp.tile([128, 128], BF16)
            make_identity(nc, ident)
            ones_b = pp.tile([128, 1], BF16)
            nc.vector.memset(ones_b, 1.0)
            ones_r = pp.tile([1, 128], BF16)   # broadcast lhsT (partition 0)
            nc.vector.memset(ones_r, 1.0)
            eps_t = pp.tile([1, 1], F32)
            nc.vector.memset(eps_t, EPS)

            def load_cols(name, dram_ap, n, pool=None, cast=False):
                t = (pool or pp).tile([128, n], F32, tag=name)
                eng = nc.gpsimd if cast else nc.sync
                eng.dma_start(
                    out=t, in_=dram_ap.rearrange("(n p) -> p n", p=128))
                return t

            projb_sb = load_cols("projb", proj_b.ap(), 6)
            fc2b_sb = load_cols("fc2b", fc2_b.ap(), 6)
            ln1w_sb = load_cols("ln1w", ln1_w.ap(), 6)
            ln2w_sb = load_cols("ln2w", ln2_w.ap(), 6)
            ln1b_sb = load_cols("ln1b", ln1_b.ap(), 6)
            ln2b_sb = load_cols("ln2b", ln2_b.ap(), 6)
            ln1b_c = pp.tile([128, KD], BF16)
            nc.vector.tensor_copy(out=ln1b_c, in_=ln1b_sb)
            ln2b_c = pp.tile([128, KD], BF16)
            nc.vector.tensor_copy(out=ln2b_c, in_=ln2b_sb)
            # LN2 stats computed in phase A, consumed in phase B
            mu2 = pp.tile([1, TOK], BF16, tag="mu2")
            rstd2 = pp.tile([1, TOK], BF16, tag="rstd2")

            # ---------------- shared helpers ----------------
            def cast_weights(dram, ktiles, ncols, dst, stg, lnw=None,
                             eng_flip=0):
                """Cast f32 weights to resident bf16; optionally fold the
                preceding layernorm's scale: W'[p,:] = lnw[p] * W[p,:]."""
                i = 0
                for k in range(ktiles):
                    for c0, cw in _chunks(ncols, 512):
                        st = stg.tile([128, 512], F32, tag="wstage", bufs=2)
                        nc.sync.dma_start(
                            out=st[:, :cw],
                            in_=dram[k * 128:(k + 1) * 128, c0:c0 + cw])
                        dd = dst[:, k, c0:c0 + cw]
                        if lnw is not None:
                            nc.vector.tensor_scalar_mul(
                                out=dd, in0=st[:, :cw],
                                scalar1=lnw[:, k:k + 1])
                        elif (i + eng_flip) % 2 == 0:
                            nc.vector.tensor_copy(out=dd, in_=st[:, :cw])
                        else:
                            nc.scalar.copy(out=dd, in_=st[:, :cw])
                        i += 1

            def fold_bias(lnb_c, w_sb, ncols, base_b, bias_dram, ps_pool,
                          rot_pool):
                """row = lnb @ W + base_b -> DRAM (then reloaded per-part.)"""
                row = rot_pool.tile([1, ncols], BF16, tag="brow", bufs=1)
                base = rot_pool.tile([1, ncols], BF16, tag="bbase", bufs=1)
                nc.gpsimd.dma_start(
                    out=base,
                    in_=base_b.rearrange("(one n) -> one n", one=1))
                for c0, cw in _chunks(ncols, 512):
                    ps = ps_pool.tile([1, 512], F32, tag="lnb")
                    for k in range(KD):
                        nc.tensor.matmul(ps[:, :cw], lnb_c[:, k:k + 1],
                                         w_sb[:, k, c0:c0 + cw],
                                         start=(k == 0), stop=(k == KD - 1))
                    nc.vector.tensor_add(out=row[:, c0:c0 + cw],
                                         in0=ps[:, :cw],
                                         in1=base[:, c0:c0 + cw])
                nc.sync.dma_start(out=bias_dram.ap().rearrange(
                    "(one n) -> one n", one=1), in_=row)

            def ln_stats(srcb, mu, rstd, chunks, ps_pool, rot_pool):
                """srcb: [128, KD, ncols] bf16 feature-major; mu/rstd: bf16
                [1, ncols] AP slices. rstd = exp(-0.5*ln(var+eps))."""
                ncols = sum(c[1] for c in chunks)
                muf = rot_pool.tile([1, 577], F32, tag="muf", bufs=1)
                msq = rot_pool.tile([1, 577], F32, tag="msq", bufs=1)
                for t0, cnt in chunks:
                    sq = rot_pool.tile([128, KD, 512], BF16, tag="sq", bufs=1)
                    nc.vector.tensor_mul(out=sq[:, :, :cnt],
                                         in0=srcb[:, :, t0:t0 + cnt],
                                         in1=srcb[:, :, t0:t0 + cnt])
                    ps_s = ps_pool.tile([1, 512], F32, tag="lnb")
                    for k in range(KD):
                        nc.tensor.matmul(ps_s[:, :cnt], ones_b,
                                         srcb[:, k, t0:t0 + cnt],
                                         start=(k == 0), stop=(k == KD - 1))
                    nc.scalar.mul(out=muf[:, t0:t0 + cnt], in_=ps_s[:, :cnt],
                                  mul=1.0 / D)
                    ps_q = ps_pool.tile([1, 512], F32, tag="lnb")
                    for k in range(KD):
                        nc.tensor.matmul(ps_q[:, :cnt], ones_b,
                                         sq[:, k, :cnt],
                                         start=(k == 0), stop=(k == KD - 1))
                    nc.scalar.mul(out=msq[:, t0:t0 + cnt], in_=ps_q[:, :cnt],
                                  mul=1.0 / D)
                var = rot_pool.tile([1, 577], F32, tag="var", bufs=1)
                nc.vector.tensor_mul(out=var[:, :ncols], in0=muf[:, :ncols],
                                     in1=muf[:, :ncols])
                nc.vector.tensor_sub(out=var[:, :ncols], in0=msq[:, :ncols],
                                     in1=var[:, :ncols])
                nc.vector.tensor_copy(out=mu, in_=muf[:, :ncols])
                nc.scalar.activation(out=var[:, :ncols], in_=var[:, :ncols],
                                     func=AF.Ln, bias=eps_t)
                nc.scalar.activation(out=rstd, in_=var[:, :ncols],
                                     func=AF.Exp, scale=-0.5)

            def ln_bcast(mu, rstd, t0, cnt, ps_pool, rot_pool):
                """Broadcast bf16 mu/rstd rows to [128, cnt] bf16."""
                ps_mu = ps_pool.tile([128, 512], F32, tag="lnb")
                nc.tensor.matmul(ps_mu[:, :cnt], ones_r, mu[:, t0:t0 + cnt],
                                 start=True, stop=True)
                mu_b = rot_pool.tile([128, 577], BF16, tag="mu_b", bufs=1)
                nc.vector.tensor_copy(out=mu_b[:, :cnt], in_=ps_mu[:, :cnt])
                ps_rs = ps_pool.tile([128, 512], F32, tag="lnb")
                nc.tensor.matmul(ps_rs[:, :cnt], ones_r, rstd[:, t0:t0 + cnt],
                                 start=True, stop=True)
                rs_b = rot_pool.tile([128, 577], BF16, tag="rs_b", bufs=1)
                nc.scalar.copy(out=rs_b[:, :cnt], in_=ps_rs[:, :cnt])
                return mu_b, rs_b

            def ln_apply(src_ap, mu_b, rs_b, dst_ap, cnt, rot_pool):
                """dst = (src - mu)*rstd, batched over k (broadcast tiles).
                LN scale/shift live in the folded weights/biases."""
                t1 = rot_pool.tile([128, KD, 577], BF16, tag="lnt1", bufs=1)
                a0, a1 = broadcast_tensor_aps(src_ap, mu_b[:, None, :cnt])
                nc.vector.tensor_sub(out=t1[:, :, :cnt], in0=a0, in1=a1)
                b0, b1 = broadcast_tensor_aps(t1[:, :, :cnt],
                                              rs_b[:, None, :cnt])
                nc.vector.tensor_mul(out=dst_ap, in0=b0, in1=b1)

            # ============ PHASE A ============
            with tc.tile_pool(name="pw", bufs=1) as pw, \
                 tc.tile_pool(name="pa", bufs=2) as pa, \
                 tc.tile_pool(name="pat", bufs=1) as pat, \
                 tc.tile_pool(name="rot", bufs=2) as rot, \
                 tc.tile_pool(name="stg", bufs=2) as stg, \
                 tc.tile_pool(name="ps_wa", bufs=3, space="PSUM") as ps_wa, \
                 tc.tile_pool(name="ps_dp", bufs=1, space="PSUM") as ps_dp, \
                 tc.tile_pool(name="ps_mm", bufs=2, space="PSUM") as ps_mm, \
                 tc.tile_pool(name="ps_ln", bufs=1, space="PSUM") as ps_ln:

                qkvw = pw.tile([128, KD, 3 * D], BF16, tag="qkvw")
                projw = pw.tile([128, KD, D], BF16, tag="projw")
                cast_weights(qkv_w.ap(), KD, 3 * D, qkvw, stg, lnw=ln1w_sb)
                cast_weights(proj_w.ap(), KD, D, projw, stg, eng_flip=1)

                # folded qkv bias: qb2 = ln1_b @ (diag(ln1_w) qkv_w) + qkv_b
                fold_bias(ln1b_c, qkvw, 3 * D, qkv_b.ap(), qb2_dram,
                          ps_ln, rot)
                qkvb_sb = load_cols("qkvb2", qb2_dram.ap(), 18, cast=True)
                vb_bc = pp.tile([128, D], BF16)
                nc.sync.dma_start(
                    out=vb_bc,
                    in_=qb2_dram.ap()[2 * D:3 * D]
                    .rearrange("(one d) -> one d", one=1).to_broadcast((128, D)))

                def front(b):
                    """Load + transpose x(b), LN1, qk + v projections."""
                    xTb = pa.tile([128, KD, T], BF16, tag="xTb")
                    for tt, (t0, cnt) in enumerate(CH_K):
                        tmb = stg.tile([128, D], BF16, tag="xstageb")
                        nc.gpsimd.dma_start(
                            out=tmb[:cnt, :],
                            in_=x_flat[b * T + t0: b * T + t0 + cnt, :])
                        for k in range(KD):
                            ps_tr = ps_mm.tile([128, 512], BF16, tag="mm")
                            nc.tensor.transpose(ps_tr[:, :cnt],
                                                tmb[:cnt, k * 128:(k + 1) * 128],
                                                ident[:cnt, :cnt])
                            if k % 2 == 0:
                                nc.scalar.copy(out=xTb[:, k, t0:t0 + cnt],
                                               in_=ps_tr[:, :cnt])
                            else:
                                nc.vector.tensor_copy(out=xTb[:, k, t0:t0 + cnt],
                                                      in_=ps_tr[:, :cnt])

                    # ---- LN1 -> yT bf16 (scale/shift folded into qkv) ----
                    mu = rot.tile([1, T], BF16, tag="mu", bufs=1)
                    rstd = rot.tile([1, T], BF16, tag="rstd", bufs=1)
                    ln_stats(xTb, mu[:, :], rstd[:, :], CH_T, ps_ln, rot)
                    yT = pa.tile([128, KD, T], BF16, tag="yT", bufs=1)
                    for t0, cnt in CH_T:
                        mu_b, rs_b = ln_bcast(mu, rstd, t0, cnt, ps_ln, rot)
                        ln_apply(xTb[:, :, t0:t0 + cnt], mu_b, rs_b,
                                 yT[:, :, t0:t0 + cnt], cnt, rot)

                    # ---- q,k (feature-major) ----
                    qkT = pa.tile([128, NH, T], BF16, tag="qkT")
                    for n in range(NH):
                        for t0, cnt in CH_T:
                            ps = ps_mm.tile([128, 512], F32, tag="mm")
                            for k in range(KD):
                                nc.tensor.matmul(ps[:, :cnt],
                                                 qkvw[:, k, n * 128:(n + 1) * 128],
                                                 yT[:, k, t0:t0 + cnt],
                                                 start=(k == 0), stop=(k == KD - 1))
                            if n % 2 == 0:
                                nc.scalar.activation(out=qkT[:, n, t0:t0 + cnt],
                                                     in_=ps[:, :cnt],
                                                     func=AF.Identity,
                                                     bias=qkvb_sb[:, n:n + 1])
                            else:
                                nc.vector.tensor_scalar_add(
                                    out=qkT[:, n, t0:t0 + cnt], in0=ps[:, :cnt],
                                    scalar1=qkvb_sb[:, n:n + 1])

                    # ---- v (token-major) ----
                    v_sb = pa.tile([128, len(CH_K), D], BF16, tag="v")
                    for tt, (t0, cnt) in enumerate(CH_K):
                        for f0, fw in ((0, 384), (384, 384)):
                            ps = ps_mm.tile([128, 512], F32, tag="mm")
                            for k in range(KD):
                                nc.tensor.matmul(ps[:cnt, :fw],
                                                 yT[:, k, t0:t0 + cnt],
                                                 qkvw[:, k, 2 * D + f0:2 * D + f0 + fw],
                                                 start=(k == 0), stop=(k == KD - 1))
                            nc.vector.tensor_add(out=v_sb[:cnt, tt, f0:f0 + fw],
                                                 in0=ps[:cnt, :fw],
                                                 in1=vb_bc[:cnt, f0:f0 + fw])
                    return xTb, qkT, v_sb

                def attention(b, xTb, qkT, v_sb):
                    """Softmax-over-heads attention + proj + residual; LN2
                    stats; stash x2 (bf16) to DRAM."""
                    xT2 = pa.tile([128, KD, T], BF16, tag="xT2", bufs=1)
                    for q0, qcnt in CH_Q:
                        wapA = [ps_wa.tile([128, 289], F32, tag="wa",
                                           name=f"wapA{b}_{q0}_{j}")
                                for j in range(3)]
                        attn_p = pat.tile([128, len(CH_K), 6, 289], BF16,
                                          tag="attnp")
                        for kt, (k0, kcnt) in enumerate(CH_K):
                            e_t = rot.tile([128, NH, 289], BF16, tag="e",
                                           bufs=2)
                            for hp in range(6):   # head pairs
                                ps2 = ps_dp.tile([128, 2, 512], F32, tag="dp")
                                for j in range(2):
                                    h = 2 * hp + j
                                    pb = (h % 2) * 64
                                    nc.tensor.matmul(
                                        ps2[:kcnt, j, :qcnt],
                                        qkT[pb:pb + 64, 6 + h // 2, k0:k0 + kcnt],
                                        qkT[pb:pb + 64, h // 2, q0:q0 + qcnt],
                                        start=True, stop=True,
                                        tile_position=(pb, 0))
                                nc.scalar.activation(
                                    out=e_t[:kcnt, 2 * hp:2 * hp + 2, :qcnt],
                                    in_=ps2[:kcnt, :, :qcnt],
                                    func=AF.Exp, scale=SCALE)
                            # ---- batched tree-sum over heads ----
                            zp6 = rot.tile([128, 6, 289], BF16, tag="zp6",
                                           bufs=1)
                            nc.vector.tensor_add(out=zp6[:kcnt, :, :qcnt],
                                                 in0=e_t[:kcnt, 0:6, :qcnt],
                                                 in1=e_t[:kcnt, 6:12, :qcnt])
                            z3 = rot.tile([128, 3, 289], BF16, tag="z3",
                                          bufs=1)
                            nc.vector.tensor_add(out=z3[:kcnt, :, :qcnt],
                                                 in0=zp6[:kcnt, 0:3, :qcnt],
                                                 in1=zp6[:kcnt, 3:6, :qcnt])
                            za = rot.tile([128, 289], BF16, tag="za", bufs=1)
                            nc.vector.tensor_add(out=za[:kcnt, :qcnt],
                                                 in0=z3[:kcnt, 0, :qcnt],
                                                 in1=z3[:kcnt, 1, :qcnt])
                            z = rot.tile([128, 289], F32, tag="z", bufs=1)
                            nc.vector.tensor_add(out=z[:kcnt, :qcnt],
                                                 in0=za[:kcnt, :qcnt],
                                                 in1=z3[:kcnt, 2, :qcnt])
                            rz = rot.tile([128, 289], F32, tag="rz", bufs=1)
                            nc.vector.reciprocal_approx_fast(
                                out=rz[:kcnt, :qcnt], in_=z[:kcnt, :qcnt])
                            rzb = rot.tile([128, 289], BF16, tag="rzb", bufs=1)
                            nc.vector.tensor_copy(out=rzb[:kcnt, :qcnt],
                                                  in_=rz[:kcnt, :qcnt])
                            # ---- normalize (broadcast over heads) ----
                            attn = rot.tile([128, 6, 289], BF16, tag="attn",
                                            bufs=1)
                            a0, a1 = broadcast_tensor_aps(
                                e_t[:kcnt, 0:6, :qcnt], rzb[:kcnt, None, :qcnt])
                            nc.vector.tensor_mul(out=attn[:kcnt, :, :qcnt],
                                                 in0=a0, in1=a1)
                            b0, b1 = broadcast_tensor_aps(
                                e_t[:kcnt, 6:12, :qcnt], rzb[:kcnt, None, :qcnt])
                            nc.vector.tensor_mul(
                                out=attn_p[:kcnt, kt, :, :qcnt], in0=b0, in1=b1)
                            for h in range(6):
                                pb = (h % 2) * 64
                                nc.tensor.matmul(
                                    wapA[h // 2][pb:pb + 64, :qcnt],
                                    v_sb[:kcnt, kt, h * 64:(h + 1) * 64],
                                    attn[:kcnt, h, :qcnt],
                                    start=(kt == 0), stop=(kt == len(CH_K) - 1),
                                    tile_position=(0, pb),
                                    skip_group_check=True)
                        waB = pat.tile([128, KD, 289], BF16, tag="waB")
                        for j in range(3):
                            nc.vector.tensor_copy(out=waB[:, j, :qcnt],
                                                  in_=wapA[j][:, :qcnt])
                        wapB = [ps_wa.tile([128, 289], F32, tag="wa",
                                           name=f"wapB{b}_{q0}_{j}")
                                for j in range(3)]
                        for kt, (k0, kcnt) in enumerate(CH_K):
                            for h in range(6, NH):
                                pb = (h % 2) * 64
                                nc.tensor.matmul(
                                    wapB[(h - 6) // 2][pb:pb + 64, :qcnt],
                                    v_sb[:kcnt, kt, h * 64:(h + 1) * 64],
                                    attn_p[:kcnt, kt, h - 6, :qcnt],
                                    start=(kt == 0), stop=(kt == len(CH_K) - 1),
                                    tile_position=(0, pb),
                                    skip_group_check=True)
                        for j in range(3):
                            nc.scalar.copy(out=waB[:, 3 + j, :qcnt],
                                           in_=wapB[j][:, :qcnt])

                        # ---- proj + residual for this q-chunk ----
                        for n in range(KD):
                            ps = ps_mm.tile([128, 512], F32, tag="mm")
                            for k in range(KD):
                                nc.tensor.matmul(ps[:, :qcnt],
                                                 projw[:, k, n * 128:(n + 1) * 128],
                                                 waB[:, k, :qcnt],
                                                 start=(k == 0), stop=(k == KD - 1))
                            nc.vector.scalar_tensor_tensor(
                                out=xT2[:, n, q0:q0 + qcnt], in0=ps[:, :qcnt],
                                scalar=projb_sb[:, n:n + 1],
                                in1=xTb[:, n, q0:q0 + qcnt],
                                op0=ALU.add, op1=ALU.add)

                    # ---- LN2 stats for this batch + stash ----
                    ln_stats(xT2, mu2[:, b * T:(b + 1) * T],
                             rstd2[:, b * T:(b + 1) * T], CH_T, ps_ln, rot)
                    for k in range(KD):
                        nc.sync.dma_start(out=x2_dram.ap()[k, :, b * T:(b + 1) * T],
                                          in_=xT2[:, k, :])

                state = front(0)
                for b in range(BL):
                    nxt = front(b + 1) if b + 1 < BL else None
                    attention(b, *state)
                    state = nxt

            # ============ PHASE B (MLP over global tokens) ============
            with tc.tile_pool(name="pwB", bufs=1) as pwB, \
                 tc.tile_pool(name="pb", bufs=2) as pb, \
                 tc.tile_pool(name="pbh", bufs=1) as pbh, \
                 tc.tile_pool(name="stgB", bufs=2) as stgB, \
                 tc.tile_pool(name="ps_mmB", bufs=3, space="PSUM") as ps_mmB, \
                 tc.tile_pool(name="ps_trB", bufs=2, space="PSUM") as ps_trB, \
                 tc.tile_pool(name="ps_lnB", bufs=1, space="PSUM") as ps_lnB:

                fc1w = pwB.tile([128, KD, HID], BF16, tag="fc1w")
                cast_weights(fc1_w.ap(), KD, HID, fc1w, stgB, lnw=ln2w_sb)
                fold_bias(ln2b_c, fc1w, HID, fc1_b.ap(), fb2_dram,
                          ps_lnB, pb)
                fc1b_sb = load_cols("fc1b2", fb2_dram.ap(), 24, cast=True)
                fc2w = pwB.tile([128, KH, D], BF16, tag="fc2w")
                cast_weights(fc2_w.ap(), KH, D, fc2w, stgB, eng_flip=1)

                for g0, cnt in CH_G:
                    x2c = pb.tile([128, KD, 512], BF16, tag="x2c")
                    for k in range(KD):
                        nc.sync.dma_start(out=x2c[:, k, :cnt],
                                          in_=x2_dram.ap()[k, :, g0:g0 + cnt])
                    mu_b, rs_b = ln_bcast(mu2, rstd2, g0, cnt, ps_lnB, pb)
                    zT = pb.tile([128, KD, 512], BF16, tag="zT")
                    ln_apply(x2c[:, :, :cnt], mu_b, rs_b, zT[:, :, :cnt],
                             cnt, pb)

                    h_t = pbh.tile([128, KH, 512], BF16, tag="h")
                    for n in range(KH):
                        ps = ps_mmB.tile([128, 512], F32, tag="mmB")
                        for k in range(KD):
                            nc.tensor.matmul(ps[:, :cnt],
                                             fc1w[:, k, n * 128:(n + 1) * 128],
                                             zT[:, k, :cnt],
                                             start=(k == 0), stop=(k == KD - 1))
                        nc.scalar.activation(out=h_t[:, n, :cnt], in_=ps[:, :cnt],
                                             func=AF.Gelu,
                                             bias=fc1b_sb[:, n:n + 1])
                    xf = pb.tile([128, KD, 512], BF16, tag="xf")
                    for n in range(KD):
                        ps = ps_mmB.tile([128, 512], F32, tag="mmB")
                        for k in range(KH):
                            nc.tensor.matmul(ps[:, :cnt],
                                             fc2w[:, k, n * 128:(n + 1) * 128],
                                             h_t[:, k, :cnt],
                                             start=(k == 0), stop=(k == KH - 1))
                        nc.vector.scalar_tensor_tensor(
                            out=xf[:, n, :cnt], in0=ps[:, :cnt],
                            scalar=fc2b_sb[:, n:n + 1],
                            in1=x2c[:, n, :cnt],
                            op0=ALU.add, op1=ALU.add)
                    # ---- transpose back to token-major and store ----
                    for c0, ccnt in _chunks(cnt, 128):
                        om = stgB.tile([128, D], BF16, tag="om")
                        for k in range(KD):
                            ps_tr = ps_trB.tile([128, 128], BF16, tag="tr")
                            nc.tensor.transpose(ps_tr[:ccnt, :],
                                                xf[:, k, c0:c0 + ccnt], ident)
                            if k % 2 == 0:
                                nc.scalar.copy(out=om[:ccnt, k * 128:(k + 1) * 128],
                                               in_=ps_tr[:ccnt, :])
                            else:
                                nc.vector.tensor_copy(
                                    out=om[:ccnt, k * 128:(k + 1) * 128],
                                    in_=ps_tr[:ccnt, :])
                        nc.gpsimd.dma_start(
                            out=out_flat[g0 + c0:g0 + c0 + ccnt, :],
                            in_=om[:ccnt, :])

    nc.compile()
    return nc


def kernel(**inputs) -> np.ndarray:
    if "nc" in _NC_CACHE:
        nc = _NC_CACHE["nc"]
    else:
        nc = _NC_CACHE["nc"] = build_nc()
    x = np.ascontiguousarray(np.asarray(inputs["x"], dtype=np.float32))
    weights = {k: np.ascontiguousarray(np.asarray(v, dtype=np.float32))
               for k, v in inputs.items() if k != "x"}
    in_maps = []
    for c in range(N_CORES):
        m = {"x": x[c * BL:(c + 1) * BL]}
        m.update(weights)
        in_maps.append(m)
    last_err = None
    for attempt in range(3):
        try:
            r = run_bass_kernel_spmd(nc, in_maps, core_ids=list(range(N_CORES)))
            return np.concatenate([r.results[c]["out"] for c in range(N_CORES)],
                                  axis=0)
        except Exception as e:  # transient device flakes: retry
            last_err = e
    raise last_err


# revision 11
# speedup vs baseline: 65.0372x; 1.0385x over previous
"""Trainium2 Bass kernel for a pre-LN transformer block (dense_transformer).

Problem shapes (hardcoded): x [32, 577, 768], 12 heads, dh=64, mlp 3072.
NOTE: softmax in the reference is over the HEADS axis (dim=1 of [B,h,T,T]),
replicated faithfully here.

Sharding: pure data-parallel over batch: 8 cores x 4 batches each.
Weights replicated. No collectives.

Layout: activations feature-major ([feature partitions, tokens free]) so
matmuls consume natural weight tiles [k,n]; matmuls in bf16 with fp32 PSUM.

Structure for PE density (HAM warmth): phase A emits batch b+1's front
(load/LN1/qkv/v) before batch b's attention so the scheduler backfills the
PE during softmax stretches; proj runs per q-chunk. Phase B (MLP) uses
LN2 stats precomputed in phase A and double-buffered token chunks.

Softmax-over-heads: paired-PSUM-bank exp, batched multi-dim DVE tree-sum,
reciprocal_approx_fast, head-broadcast normalize.

LN scale/shift are folded into the following matmul: W' = diag(w) @ W at
weight-cast time and b' = b_ln @ W + b at kernel start (tiny PE matmuls +
a DRAM roundtrip to relayout the folded bias row per-partition).
"""
import numpy as np

import concourse.bacc as bacc
import concourse.mybir as mybir
import concourse.tile as tile
from concourse.bass import broadcast_tensor_aps
from concourse.bass_utils import run_bass_kernel_spmd
from concourse.masks import make_identity

F32 = mybir.dt.float32
BF16 = mybir.dt.bfloat16
AF = mybir.ActivationFunctionType
ALU = mybir.AluOpType

N_CORES = 8
B, T, D = 32, 577, 768
BL = B // N_CORES          # 4 batches per core
NH, DH = 12, 64            # heads
HID = 4 * D                # 3072
KD = D // 128              # 6 feature tiles
KH = HID // 128            # 24 hidden tiles
EPS = 1e-6
SCALE = DH ** -0.5

TOK = BL * T               # 2308 tokens per core
CH_T = [(0, 512), (512, 65)]                      # within one batch (577)
CH_Q = [(0, 289), (289, 288)]                     # attention q chunks
CH_K = [(0, 128), (128, 128), (256, 128), (384, 128), (512, 65)]  # kt tiles
CH_G = [(0, 512), (512, 512), (1024, 512), (1536, 512), (2048, 260)]  # global

_NC_CACHE = {}


def _chunks(total, step):
    out = []
    o = 0
    while o < total:
        out.append((o, min(step, total - o)))
        o += step
    return out


def build_nc():
    nc = bacc.Bacc("TRN2")
    x = nc.dram_tensor("x", [BL, T, D], F32, kind="ExternalInput")
    ln1_w = nc.dram_tensor("ln1_w", [D], F32, kind="ExternalInput")
    ln1_b = nc.dram_tensor("ln1_b", [D], F32, kind="ExternalInput")
    qkv_w = nc.dram_tensor("qkv_w", [D, 3 * D], F32, kind="ExternalInput")
    qkv_b = nc.dram_tensor("qkv_b", [3 * D], F32, kind="ExternalInput")
    proj_w = nc.dram_tensor("proj_w", [D, D], F32, kind="ExternalInput")
    proj_b = nc.dram_tensor("proj_b", [D], F32, kind="ExternalInput")
    ln2_w = nc.dram_tensor("ln2_w", [D], F32, kind="ExternalInput")
    ln2_b = nc.dram_tensor("ln2_b", [D], F32, kind="ExternalInput")
    fc1_w = nc.dram_tensor("fc1_w", [D, HID], F32, kind="ExternalInput")
    fc1_b = nc.dram_tensor("fc1_b", [HID], F32, kind="ExternalInput")
    fc2_w = nc.dram_tensor("fc2_w", [HID, D], F32, kind="ExternalInput")
    fc2_b = nc.dram_tensor("fc2_b", [D], F32, kind="ExternalInput")
    out = nc.dram_tensor("out", [BL, T, D], F32, kind="ExternalOutput")

    # DRAM scratch
    x2_dram = nc.dram_tensor("x2_dram", [KD, 128, TOK], BF16, kind="Internal")
    qb2_dram = nc.dram_tensor("qb2_dram", [3 * D], BF16, kind="Internal")
    fb2_dram = nc.dram_tensor("fb2_dram", [HID], BF16, kind="Internal")

    x_flat = x.ap().rearrange("b t d -> (b t) d")      # [2308, 768]
    out_flat = out.ap().rearrange("b t d -> (b t) d")

    with tile.TileContext(nc) as tc:
        with tc.tile_pool(name="persist", bufs=1) as pp:
            ident = pp.tile([128, 128], BF16)
            make_identity(nc, ident)
            ones_b = pp.tile([128, 1], BF16)
            nc.vector.memset(ones_b, 1.0)
            ones_r = pp.tile([1, 128], BF16)   # broadcast lhsT (partition 0)
            nc.vector.memset(ones_r, 1.0)
            eps_t = pp.tile([1, 1], F32)
            nc.vector.memset(eps_t, EPS)

            def load_cols(name, dram_ap, n, pool=None, cast=False):
                t = (pool or pp).tile([128, n], F32, tag=name)
                eng = nc.gpsimd if cast else nc.sync
                eng.dma_start(
                    out=t, in_=dram_ap.rearrange("(n p) -> p n", p=128))
                return t

            projb_sb = load_cols("projb", proj_b.ap(), 6)
            fc2b_sb = load_cols("fc2b", fc2_b.ap(), 6)
            ln1w_sb = load_cols("ln1w", ln1_w.ap(), 6)
            ln2w_sb = load_cols("ln2w", ln2_w.ap(), 6)
            ln1b_sb = load_cols("ln1b", ln1_b.ap(), 6)
            ln2b_sb = load_cols("ln2b", ln2_b.ap(), 6)
            ln1b_c = pp.tile([128, KD], BF16)
            nc.vector.tensor_copy(out=ln1b_c, in_=ln1b_sb)
            ln2b_c = pp.tile([128, KD], BF16)
            nc.vector.tensor_copy(out=ln2b_c, in_=ln2b_sb)
            # LN2 stats computed in phase A, consumed in phase B
            mu2 = pp.tile([1, TOK], BF16, tag="mu2")
            rstd2 = pp.tile([1, TOK], BF16, tag="rstd2")

            # ---------------- shared helpers ----------------
            def cast_weights(dram, ktiles, ncols, dst, stg, lnw=None,
                             eng_flip=0):
                """Cast f32 weights to resident bf16, n-major (chunk-outer)
                so consumers of early n-tiles start before the full load;
                optionally fold the preceding layernorm's scale:
                W'[p,:] = lnw[p] * W[p,:]."""
                i = 0
                for c0, cw in _chunks(ncols, 512):
                    for k in range(ktiles):
                        st = stg.tile([128, 512], F32, tag="wstage", bufs=2)
                        nc.sync.dma_start(
                            out=st[:, :cw],
                            in_=dram[k * 128:(k + 1) * 128, c0:c0 + cw])
                        dd = dst[:, k, c0:c0 + cw]
                        if lnw is not None:
                            nc.vector.tensor_scalar_mul(
                                out=dd, in0=st[:, :cw],
                                scalar1=lnw[:, k:k + 1])
                        elif (i + eng_flip) % 2 == 0:
                            nc.vector.tensor_copy(out=dd, in_=st[:, :cw])
                        else:
                            nc.scalar.copy(out=dd, in_=st[:, :cw])
                        i += 1

            def fold_bias(lnb_c, w_sb, ncols, base_b, bias_dram, dst_sb,
                          ps_pool, rot_pool):
                """Per 512-chunk: row_c = lnb @ W_c + base_c -> DRAM ->
                reload as per-partition bias columns of dst_sb."""
                row = rot_pool.tile([1, ncols], BF16, tag="brow", bufs=1)
                base = rot_pool.tile([1, ncols], BF16, tag="bbase", bufs=1)
                nc.gpsimd.dma_start(
                    out=base,
                    in_=base_b.rearrange("(one n) -> one n", one=1))
                for c0, cw in _chunks(ncols, 512):
                    ps = ps_pool.tile([1, 512], F32, tag="mm")
                    for k in range(KD):
                        nc.tensor.matmul(ps[:, :cw], lnb_c[:, k:k + 1],
                                         w_sb[:, k, c0:c0 + cw],
                                         start=(k == 0), stop=(k == KD - 1))
                    nc.vector.tensor_add(out=row[:, c0:c0 + cw],
                                         in0=ps[:, :cw],
                                         in1=base[:, c0:c0 + cw])
                    nc.sync.dma_start(
                        out=bias_dram.ap()[c0:c0 + cw].rearrange(
                            "(one n) -> one n", one=1),
                        in_=row[:, c0:c0 + cw])
                    nc.gpsimd.dma_start(
                        out=dst_sb[:, c0 // 128:(c0 + cw) // 128],
                        in_=bias_dram.ap()[c0:c0 + cw].rearrange(
                            "(n p) -> p n", p=128))

            def ln_stats(srcb, mu, rstd, chunks, ps_pool, rot_pool):
                """srcb: [128, KD, ncols] bf16 feature-major; mu/rstd: bf16
                [1, ncols] AP slices. rstd = exp(-0.5*ln(var+eps))."""
                ncols = sum(c[1] for c in chunks)
                muf = rot_pool.tile([1, 577], F32, tag="muf", bufs=1)
                msq = rot_pool.tile([1, 577], F32, tag="msq", bufs=1)
                for t0, cnt in chunks:
                    sq = rot_pool.tile([128, KD, 512], BF16, tag="sq", bufs=1)
                    nc.vector.tensor_mul(out=sq[:, :, :cnt],
                                         in0=srcb[:, :, t0:t0 + cnt],
                                         in1=srcb[:, :, t0:t0 + cnt])
                    ps_s = ps_pool.tile([1, 512], F32, tag="mm")
                    for k in range(KD):
                        nc.tensor.matmul(ps_s[:, :cnt], ones_b,
                                         srcb[:, k, t0:t0 + cnt],
                                         start=(k == 0), stop=(k == KD - 1))
                    nc.scalar.mul(out=muf[:, t0:t0 + cnt], in_=ps_s[:, :cnt],
                                  mul=1.0 / D)
                    ps_q = ps_pool.tile([1, 512], F32, tag="mm")
                    for k in range(KD):
                        nc.tensor.matmul(ps_q[:, :cnt], ones_b,
                                         sq[:, k, :cnt],
                                         start=(k == 0), stop=(k == KD - 1))
                    nc.scalar.mul(out=msq[:, t0:t0 + cnt], in_=ps_q[:, :cnt],
                                  mul=1.0 / D)
                var = rot_pool.tile([1, 577], F32, tag="var", bufs=1)
                nc.vector.tensor_mul(out=var[:, :ncols], in0=muf[:, :ncols],
                                     in1=muf[:, :ncols])
                nc.vector.tensor_sub(out=var[:, :ncols], in0=msq[:, :ncols],
                                     in1=var[:, :ncols])
                nc.vector.tensor_copy(out=mu, in_=muf[:, :ncols])
                nc.scalar.activation(out=var[:, :ncols], in_=var[:, :ncols],
                                     func=AF.Ln, bias=eps_t)
                nc.scalar.activation(out=rstd, in_=var[:, :ncols],
                                     func=AF.Exp, scale=-0.5)

            def ln_bcast(mu, rstd, t0, cnt, ps_pool, rot_pool):
                """Broadcast bf16 mu/rstd rows to [128, cnt] bf16."""
                ps_mu = ps_pool.tile([128, 512], F32, tag="mm")
                nc.tensor.matmul(ps_mu[:, :cnt], ones_r, mu[:, t0:t0 + cnt],
                                 start=True, stop=True)
                mu_b = rot_pool.tile([128, 577], BF16, tag="mu_b", bufs=1)
                nc.vector.tensor_copy(out=mu_b[:, :cnt], in_=ps_mu[:, :cnt])
                ps_rs = ps_pool.tile([128, 512], F32, tag="mm")
                nc.tensor.matmul(ps_rs[:, :cnt], ones_r, rstd[:, t0:t0 + cnt],
                                 start=True, stop=True)
                rs_b = rot_pool.tile([128, 577], BF16, tag="rs_b", bufs=1)
                nc.scalar.copy(out=rs_b[:, :cnt], in_=ps_rs[:, :cnt])
                return mu_b, rs_b

            def ln_apply(src_ap, mu_b, rs_b, dst_ap, cnt, rot_pool):
                """dst = (src - mu)*rstd, batched over k (broadcast tiles).
                LN scale/shift live in the folded weights/biases."""
                t1 = rot_pool.tile([128, KD, 577], BF16, tag="lnt1", bufs=1)
                a0, a1 = broadcast_tensor_aps(src_ap, mu_b[:, None, :cnt])
                nc.vector.tensor_sub(out=t1[:, :, :cnt], in0=a0, in1=a1)
                b0, b1 = broadcast_tensor_aps(t1[:, :, :cnt],
                                              rs_b[:, None, :cnt])
                nc.vector.tensor_mul(out=dst_ap, in0=b0, in1=b1)

            # ============ PHASE A ============
            with tc.tile_pool(name="pw", bufs=1) as pw, \
                 tc.tile_pool(name="pa", bufs=2) as pa, \
                 tc.tile_pool(name="pat", bufs=1) as pat, \
                 tc.tile_pool(name="rot", bufs=2) as rot, \
                 tc.tile_pool(name="stg", bufs=2) as stg, \
                 tc.tile_pool(name="ps_wa", bufs=3, space="PSUM") as ps_wa, \
                 tc.tile_pool(name="ps_dp", bufs=3, space="PSUM") as ps_dp, \
                 tc.tile_pool(name="ps_mm", bufs=2, space="PSUM") as ps_mm:

                qkvw = pw.tile([128, KD, 3 * D], BF16, tag="qkvw")
                projw = pw.tile([128, KD, D], BF16, tag="projw")
                cast_weights(qkv_w.ap(), KD, 3 * D, qkvw, stg, lnw=ln1w_sb)
                cast_weights(proj_w.ap(), KD, D, projw, stg, eng_flip=1)

                # folded qkv bias: qb2 = ln1_b @ (diag(ln1_w) qkv_w) + qkv_b
                qkvb_sb = pp.tile([128, 18], F32, tag="qkvb2")
                fold_bias(ln1b_c, qkvw, 3 * D, qkv_b.ap(), qb2_dram,
                          qkvb_sb, ps_mm, rot)
                vb_bc = pp.tile([128, D], BF16)
                nc.sync.dma_start(
                    out=vb_bc,
                    in_=qb2_dram.ap()[2 * D:3 * D]
                    .rearrange("(one d) -> one d", one=1).to_broadcast((128, D)))

                def front(b):
                    """Load + transpose x(b), LN1, qk + v projections."""
                    xTb = pa.tile([128, KD, T], BF16, tag="xTb")
                    for tt, (t0, cnt) in enumerate(CH_K):
                        tmb = stg.tile([128, D], BF16, tag="xstageb")
                        nc.gpsimd.dma_start(
                            out=tmb[:cnt, :],
                            in_=x_flat[b * T + t0: b * T + t0 + cnt, :])
                        for k in range(KD):
                            ps_tr = ps_mm.tile([128, 512], BF16, tag="mm")
                            nc.tensor.transpose(ps_tr[:, :cnt],
                                                tmb[:cnt, k * 128:(k + 1) * 128],
                                                ident[:cnt, :cnt])
                            if k % 2 == 0:
                                nc.scalar.copy(out=xTb[:, k, t0:t0 + cnt],
                                               in_=ps_tr[:, :cnt])
                            else:
                                nc.vector.tensor_copy(out=xTb[:, k, t0:t0 + cnt],
                                                      in_=ps_tr[:, :cnt])

                    # ---- LN1 -> yT bf16 (scale/shift folded into qkv) ----
                    mu = rot.tile([1, T], BF16, tag="mu", bufs=1)
                    rstd = rot.tile([1, T], BF16, tag="rstd", bufs=1)
                    ln_stats(xTb, mu[:, :], rstd[:, :], CH_T, ps_mm, rot)
                    yT = pa.tile([128, KD, T], BF16, tag="yT", bufs=1)
                    for t0, cnt in CH_T:
                        mu_b, rs_b = ln_bcast(mu, rstd, t0, cnt, ps_mm, rot)
                        ln_apply(xTb[:, :, t0:t0 + cnt], mu_b, rs_b,
                                 yT[:, :, t0:t0 + cnt], cnt, rot)

                    # ---- q,k (feature-major) ----
                    qkT = pa.tile([128, NH, T], BF16, tag="qkT")
                    for n in range(NH):
                        for t0, cnt in CH_T:
                            ps = ps_mm.tile([128, 512], F32, tag="mm")
                            for k in range(KD):
                                nc.tensor.matmul(ps[:, :cnt],
                                                 qkvw[:, k, n * 128:(n + 1) * 128],
                                                 yT[:, k, t0:t0 + cnt],
                                                 start=(k == 0), stop=(k == KD - 1))
                            if n % 2 == 0:
                                nc.scalar.activation(out=qkT[:, n, t0:t0 + cnt],
                                                     in_=ps[:, :cnt],
                                                     func=AF.Identity,
                                                     bias=qkvb_sb[:, n:n + 1])
                            else:
                                nc.vector.tensor_scalar_add(
                                    out=qkT[:, n, t0:t0 + cnt], in0=ps[:, :cnt],
                                    scalar1=qkvb_sb[:, n:n + 1])

                    # ---- v (token-major) ----
                    v_sb = pa.tile([128, len(CH_K), D], BF16, tag="v")
                    for tt, (t0, cnt) in enumerate(CH_K):
                        for f0, fw in ((0, 384), (384, 384)):
                            ps = ps_mm.tile([128, 512], F32, tag="mm")
                            for k in range(KD):
                                nc.tensor.matmul(ps[:cnt, :fw],
                                                 yT[:, k, t0:t0 + cnt],
                                                 qkvw[:, k, 2 * D + f0:2 * D + f0 + fw],
                                                 start=(k == 0), stop=(k == KD - 1))
                            nc.vector.tensor_add(out=v_sb[:cnt, tt, f0:f0 + fw],
                                                 in0=ps[:cnt, :fw],
                                                 in1=vb_bc[:cnt, f0:f0 + fw])
                    return xTb, qkT, v_sb

                def attention(b, xTb, qkT, v_sb):
                    """Softmax-over-heads attention + proj + residual; LN2
                    stats; stash x2 (bf16) to DRAM."""
                    xT2 = pa.tile([128, KD, T], BF16, tag="xT2", bufs=1)
                    for q0, qcnt in CH_Q:
                        wapA = [ps_wa.tile([128, 289], F32, tag="wa",
                                           name=f"wapA{b}_{q0}_{j}")
                                for j in range(3)]
                        attn_p = pat.tile([128, len(CH_K), 6, 289], BF16,
                                          tag="attnp")
                        for kt, (k0, kcnt) in enumerate(CH_K):
                            e_t = rot.tile([128, NH, 289], BF16, tag="e",
                                           bufs=2)
                            for h in range(NH):
                                pb = (h % 2) * 64
                                ps1 = ps_dp.tile([128, 512], F32, tag="dp")
                                nc.tensor.matmul(
                                    ps1[:kcnt, :qcnt],
                                    qkT[pb:pb + 64, 6 + h // 2, k0:k0 + kcnt],
                                    qkT[pb:pb + 64, h // 2, q0:q0 + qcnt],
                                    start=True, stop=True,
                                    tile_position=(pb, 0))
                                nc.scalar.activation(
                                    out=e_t[:kcnt, h, :qcnt],
                                    in_=ps1[:kcnt, :qcnt],
                                    func=AF.Exp, scale=SCALE)
                            # ---- batched tree-sum over heads ----
                            zp6 = rot.tile([128, 6, 289], BF16, tag="zp6",
                                           bufs=1)
                            nc.vector.tensor_add(out=zp6[:kcnt, :, :qcnt],
                                                 in0=e_t[:kcnt, 0:6, :qcnt],
                                                 in1=e_t[:kcnt, 6:12, :qcnt])
                            z3 = rot.tile([128, 3, 289], BF16, tag="z3",
                                          bufs=1)
                            nc.vector.tensor_add(out=z3[:kcnt, :, :qcnt],
                                                 in0=zp6[:kcnt, 0:3, :qcnt],
                                                 in1=zp6[:kcnt, 3:6, :qcnt])
                            za = rot.tile([128, 289], BF16, tag="za", bufs=1)
                            nc.vector.tensor_add(out=za[:kcnt, :qcnt],
                                                 in0=z3[:kcnt, 0, :qcnt],
                                                 in1=z3[:kcnt, 1, :qcnt])
                            z = rot.tile([128, 289], F32, tag="z", bufs=1)
                            nc.vector.tensor_add(out=z[:kcnt, :qcnt],
                                                 in0=za[:kcnt, :qcnt],
                                                 in1=z3[:kcnt, 2, :qcnt])
                            rz = rot.tile([128, 289], F32, tag="rz", bufs=1)
                            nc.vector.reciprocal_approx_fast(
                                out=rz[:kcnt, :qcnt], in_=z[:kcnt, :qcnt])
                            rzb = rot.tile([128, 289], BF16, tag="rzb", bufs=1)
                            nc.vector.tensor_copy(out=rzb[:kcnt, :qcnt],
                                                  in_=rz[:kcnt, :qcnt])
                            # ---- normalize (broadcast over heads) ----
                            attn = rot.tile([128, 6, 289], BF16, tag="attn",
                                            bufs=1)
                            a0, a1 = broadcast_tensor_aps(
                                e_t[:kcnt, 0:6, :qcnt], rzb[:kcnt, None, :qcnt])
                            nc.vector.tensor_mul(out=attn[:kcnt, :, :qcnt],
                                                 in0=a0, in1=a1)
                            b0, b1 = broadcast_tensor_aps(
                                e_t[:kcnt, 6:12, :qcnt], rzb[:kcnt, None, :qcnt])
                            nc.vector.tensor_mul(
                                out=attn_p[:kcnt, kt, :, :qcnt], in0=b0, in1=b1)
                            for h in range(6):
                                pb = (h % 2) * 64
                                nc.tensor.matmul(
                                    wapA[h // 2][pb:pb + 64, :qcnt],
                                    v_sb[:kcnt, kt, h * 64:(h + 1) * 64],
                                    attn[:kcnt, h, :qcnt],
                                    start=(kt == 0), stop=(kt == len(CH_K) - 1),
                                    tile_position=(0, pb),
                                    skip_group_check=True)
                        waB = pat.tile([128, KD, 289], BF16, tag="waB")
                        for j in range(3):
                            nc.vector.tensor_copy(out=waB[:, j, :qcnt],
                                                  in_=wapA[j][:, :qcnt])
                        wapB = [ps_wa.tile([128, 289], F32, tag="wa",
                                           name=f"wapB{b}_{q0}_{j}")
                                for j in range(3)]
                        for kt, (k0, kcnt) in enumerate(CH_K):
                            for h in range(6, NH):
                                pb = (h % 2) * 64
                                nc.tensor.matmul(
                                    wapB[(h - 6) // 2][pb:pb + 64, :qcnt],
                                    v_sb[:kcnt, kt, h * 64:(h + 1) * 64],
                                    attn_p[:kcnt, kt, h - 6, :qcnt],
                                    start=(kt == 0), stop=(kt == len(CH_K) - 1),
                                    tile_position=(0, pb),
                                    skip_group_check=True)
                        for j in range(3):
                            nc.scalar.copy(out=waB[:, 3 + j, :qcnt],
                                           in_=wapB[j][:, :qcnt])

                        # ---- proj + residual for this q-chunk ----
                        for n in range(KD):
                            ps = ps_mm.tile([128, 512], F32, tag="mm")
                            for k in range(KD):
                                nc.tensor.matmul(ps[:, :qcnt],
                                                 projw[:, k, n * 128:(n + 1) * 128],
                                                 waB[:, k, :qcnt],
                                                 start=(k == 0), stop=(k == KD - 1))
                            nc.vector.scalar_tensor_tensor(
                                out=xT2[:, n, q0:q0 + qcnt], in0=ps[:, :qcnt],
                                scalar=projb_sb[:, n:n + 1],
                                in1=xTb[:, n, q0:q0 + qcnt],
                                op0=ALU.add, op1=ALU.add)

                    # ---- LN2 stats for this batch + stash ----
                    ln_stats(xT2, mu2[:, b * T:(b + 1) * T],
                             rstd2[:, b * T:(b + 1) * T], CH_T, ps_mm, rot)
                    for k in range(KD):
                        nc.sync.dma_start(out=x2_dram.ap()[k, :, b * T:(b + 1) * T],
                                          in_=xT2[:, k, :])

                state = front(0)
                for b in range(BL):
                    nxt = front(b + 1) if b + 1 < BL else None
                    attention(b, *state)
                    state = nxt

            # ============ PHASE B (MLP over global tokens) ============
            with tc.tile_pool(name="pwB", bufs=1) as pwB, \
                 tc.tile_pool(name="pb", bufs=2) as pb, \
                 tc.tile_pool(name="pbh", bufs=1) as pbh, \
                 tc.tile_pool(name="stgB", bufs=2) as stgB, \
                 tc.tile_pool(name="ps_mmB", bufs=3, space="PSUM") as ps_mmB, \
                 tc.tile_pool(name="ps_trB", bufs=2, space="PSUM") as ps_trB:

                fc1w = pwB.tile([128, KD, HID], BF16, tag="fc1w")
                cast_weights(fc1_w.ap(), KD, HID, fc1w, stgB, lnw=ln2w_sb)
                fc1b_sb = pp.tile([128, 24], F32, tag="fc1b2")
                fold_bias(ln2b_c, fc1w, HID, fc1_b.ap(), fb2_dram,
                          fc1b_sb, ps_mmB, pb)
                fc2w = pwB.tile([128, KH, D], BF16, tag="fc2w")
                cast_weights(fc2_w.ap(), KH, D, fc2w, stgB, eng_flip=1)

                for g0, cnt in CH_G:
                    x2c = pb.tile([128, KD, 512], BF16, tag="x2c")
                    for k in range(KD):
                        nc.sync.dma_start(out=x2c[:, k, :cnt],
                                          in_=x2_dram.ap()[k, :, g0:g0 + cnt])
                    mu_b, rs_b = ln_bcast(mu2, rstd2, g0, cnt, ps_mmB, pb)
                    zT = pb.tile([128, KD, 512], BF16, tag="zT")
                    ln_apply(x2c[:, :, :cnt], mu_b, rs_b, zT[:, :, :cnt],
                             cnt, pb)

                    h_t = pbh.tile([128, KH, 512], BF16, tag="h")
                    for n in range(KH):
                        ps = ps_mmB.tile([128, 512], F32, tag="mmB")
                        for k in range(KD):
                            nc.tensor.matmul(ps[:, :cnt],
                                             fc1w[:, k, n * 128:(n + 1) * 128],
                                             zT[:, k, :cnt],
                                             start=(k == 0), stop=(k == KD - 1))
                        nc.scalar.activation(out=h_t[:, n, :cnt], in_=ps[:, :cnt],
                                             func=AF.Gelu,
                                             bias=fc1b_sb[:, n:n + 1])
                    xf = pb.tile([128, KD, 512], BF16, tag="xf")
                    for n in range(KD):
                        ps = ps_mmB.tile([128, 512], F32, tag="mmB")
                        for k in range(KH):
                            nc.tensor.matmul(ps[:, :cnt],
                                             fc2w[:, k, n * 128:(n + 1) * 128],
                                             h_t[:, k, :cnt],
                                             start=(k == 0), stop=(k == KH - 1))
                        nc.vector.scalar_tensor_tensor(
                            out=xf[:, n, :cnt], in0=ps[:, :cnt],
                            scalar=fc2b_sb[:, n:n + 1],
                            in1=x2c[:, n, :cnt],
                            op0=ALU.add, op1=ALU.add)
                    # ---- transpose back to token-major and store ----
                    for c0, ccnt in _chunks(cnt, 128):
                        om = stgB.tile([128, D], BF16, tag="om")
                        for k in range(KD):
                            ps_tr = ps_trB.tile([128, 128], BF16, tag="tr")
                            nc.tensor.transpose(ps_tr[:ccnt, :],
                                                xf[:, k, c0:c0 + ccnt], ident)
                            if k % 2 == 0:
                                nc.scalar.copy(out=om[:ccnt, k * 128:(k + 1) * 128],
                                               in_=ps_tr[:ccnt, :])
                            else:
                                nc.vector.tensor_copy(
                                    out=om[:ccnt, k * 128:(k + 1) * 128],
                                    in_=ps_tr[:ccnt, :])
                        nc.gpsimd.dma_start(
                            out=out_flat[g0 + c0:g0 + c0 + ccnt, :],
                            in_=om[:ccnt, :])

    nc.compile()
    return nc


def kernel(**inputs) -> np.ndarray:
    if "nc" in _NC_CACHE:
        nc = _NC_CACHE["nc"]
    else:
        nc = _NC_CACHE["nc"] = build_nc()
    x = np.ascontiguousarray(np.asarray(inputs["x"], dtype=np.float32))
    weights = {k: np.ascontiguousarray(np.asarray(v, dtype=np.float32))
               for k, v in inputs.items() if k != "x"}
    in_maps = []
    for c in range(N_CORES):
        m = {"x": x[c * BL:(c + 1) * BL]}
        m.update(weights)
        in_maps.append(m)
    last_err = None
    for attempt in range(3):
        try:
            r = run_bass_kernel_spmd(nc, in_maps, core_ids=list(range(N_CORES)))
            return np.concatenate([r.results[c]["out"] for c in range(N_CORES)],
                                  axis=0)
        except Exception as e:  # transient device flakes: retry
            last_err = e
    raise last_err


# revision 13
# speedup vs baseline: 67.8685x; 1.0435x over previous
"""Trainium2 Bass kernel for a pre-LN transformer block (dense_transformer).

Problem shapes (hardcoded): x [32, 577, 768], 12 heads, dh=64, mlp 3072.
NOTE: softmax in the reference is over the HEADS axis (dim=1 of [B,h,T,T]),
replicated faithfully here.

Sharding: pure data-parallel over batch: 8 cores x 4 batches each.
Weights replicated. No collectives.

Layout: activations feature-major ([feature partitions, tokens free]) so
matmuls consume natural weight tiles [k,n]; matmuls in bf16 with fp32 PSUM.

Structure for PE density (HAM warmth): phase A emits batch b+1's front
(load/LN1/qkv/v) before batch b's attention so the scheduler backfills the
PE during softmax stretches; proj runs per q-chunk. Phase B (MLP) uses
LN2 stats precomputed in phase A and double-buffered token chunks.

Softmax-over-heads: paired-PSUM-bank exp, batched multi-dim DVE tree-sum,
reciprocal_approx_fast, head-broadcast normalize.

LN scale/shift are folded into the following matmul: W' = diag(w) @ W at
weight-cast time and b' = b_ln @ W + b at kernel start (tiny PE matmuls +
a DRAM roundtrip to relayout the folded bias row per-partition).
"""
import numpy as np

import concourse.bacc as bacc
import concourse.mybir as mybir
import concourse.tile as tile
from concourse.bass import broadcast_tensor_aps
from concourse.bass_utils import run_bass_kernel_spmd
from concourse.masks import make_identity

F32 = mybir.dt.float32
BF16 = mybir.dt.bfloat16
AF = mybir.ActivationFunctionType
ALU = mybir.AluOpType

N_CORES = 8
B, T, D = 32, 577, 768
BL = B // N_CORES          # 4 batches per core
NH, DH = 12, 64            # heads
HID = 4 * D                # 3072
KD = D // 128              # 6 feature tiles
KH = HID // 128            # 24 hidden tiles
EPS = 1e-6
SCALE = DH ** -0.5

TOK = BL * T               # 2308 tokens per core
CH_T = [(0, 512), (512, 65)]                      # within one batch (577)
CH_Q = [(0, 289), (289, 288)]                     # attention q chunks
CH_K = [(0, 128), (128, 128), (256, 128), (384, 128), (512, 65)]  # kt tiles
CH_G = [(0, 512), (512, 512), (1024, 512), (1536, 512), (2048, 260)]  # global

_NC_CACHE = {}


def _chunks(total, step):
    out = []
    o = 0
    while o < total:
        out.append((o, min(step, total - o)))
        o += step
    return out


def build_nc():
    nc = bacc.Bacc("TRN2")
    x = nc.dram_tensor("x", [BL, T, D], F32, kind="ExternalInput")
    ln1_w = nc.dram_tensor("ln1_w", [D], F32, kind="ExternalInput")
    ln1_b = nc.dram_tensor("ln1_b", [D], F32, kind="ExternalInput")
    qkv_w = nc.dram_tensor("qkv_w", [D, 3 * D], F32, kind="ExternalInput")
    qkv_b = nc.dram_tensor("qkv_b", [3 * D], F32, kind="ExternalInput")
    proj_w = nc.dram_tensor("proj_w", [D, D], F32, kind="ExternalInput")
    proj_b = nc.dram_tensor("proj_b", [D], F32, kind="ExternalInput")
    ln2_w = nc.dram_tensor("ln2_w", [D], F32, kind="ExternalInput")
    ln2_b = nc.dram_tensor("ln2_b", [D], F32, kind="ExternalInput")
    fc1_w = nc.dram_tensor("fc1_w", [D, HID], F32, kind="ExternalInput")
    fc1_b = nc.dram_tensor("fc1_b", [HID], F32, kind="ExternalInput")
    fc2_w = nc.dram_tensor("fc2_w", [HID, D], F32, kind="ExternalInput")
    fc2_b = nc.dram_tensor("fc2_b", [D], F32, kind="ExternalInput")
    out = nc.dram_tensor("out", [BL, T, D], F32, kind="ExternalOutput")

    # DRAM scratch
    x2_dram = nc.dram_tensor("x2_dram", [KD, 128, TOK], BF16, kind="Internal")
    fc1w_dram = nc.dram_tensor("fc1w_dram", [KD, 128, HID], BF16, kind="Internal")
    fc2w_dram = nc.dram_tensor("fc2w_dram", [KH, 128, D], BF16, kind="Internal")
    qb2_dram = nc.dram_tensor("qb2_dram", [3 * D], BF16, kind="Internal")
    fb2_dram = nc.dram_tensor("fb2_dram", [HID], BF16, kind="Internal")

    x_flat = x.ap().rearrange("b t d -> (b t) d")      # [2308, 768]
    out_flat = out.ap().rearrange("b t d -> (b t) d")

    with tile.TileContext(nc) as tc:
        with tc.tile_pool(name="persist", bufs=1) as pp:
            ident = pp.tile([128, 128], BF16)
            make_identity(nc, ident)
            ones_b = pp.tile([128, 1], BF16)
            nc.vector.memset(ones_b, 1.0)
            ones_r = pp.tile([1, 128], BF16)   # broadcast lhsT (partition 0)
            nc.vector.memset(ones_r, 1.0)
            eps_t = pp.tile([1, 1], F32)
            nc.vector.memset(eps_t, EPS)

            def load_cols(name, dram_ap, n, pool=None, cast=False):
                t = (pool or pp).tile([128, n], F32, tag=name)
                eng = nc.gpsimd if cast else nc.sync
                eng.dma_start(
                    out=t, in_=dram_ap.rearrange("(n p) -> p n", p=128))
                return t

            projb_sb = load_cols("projb", proj_b.ap(), 6)
            fc2b_sb = load_cols("fc2b", fc2_b.ap(), 6)
            ln1w_sb = load_cols("ln1w", ln1_w.ap(), 6)
            ln2w_sb = load_cols("ln2w", ln2_w.ap(), 6)
            ln1b_sb = load_cols("ln1b", ln1_b.ap(), 6)
            ln2b_sb = load_cols("ln2b", ln2_b.ap(), 6)
            ln1b_c = pp.tile([128, KD], BF16)
            nc.vector.tensor_copy(out=ln1b_c, in_=ln1b_sb)
            ln2b_c = pp.tile([128, KD], BF16)
            nc.vector.tensor_copy(out=ln2b_c, in_=ln2b_sb)
            # LN2 stats computed in phase A, consumed in phase B
            mu2 = pp.tile([1, TOK], BF16, tag="mu2")
            rstd2 = pp.tile([1, TOK], BF16, tag="rstd2")

            # ---------------- shared helpers ----------------
            def cast_weights(dram, ktiles, ncols, dst, stg, lnw=None,
                             eng_flip=0):
                """Cast f32 weights to resident bf16, n-major (chunk-outer)
                so consumers of early n-tiles start before the full load;
                optionally fold the preceding layernorm's scale:
                W'[p,:] = lnw[p] * W[p,:]."""
                i = 0
                for c0, cw in _chunks(ncols, 512):
                    for k in range(ktiles):
                        st = stg.tile([128, 512], F32, tag="wstage", bufs=2)
                        nc.sync.dma_start(
                            out=st[:, :cw],
                            in_=dram[k * 128:(k + 1) * 128, c0:c0 + cw])
                        dd = dst[:, k, c0:c0 + cw]
                        if lnw is not None:
                            nc.vector.tensor_scalar_mul(
                                out=dd, in0=st[:, :cw],
                                scalar1=lnw[:, k:k + 1])
                        elif (i + eng_flip) % 2 == 0:
                            nc.vector.tensor_copy(out=dd, in_=st[:, :cw])
                        else:
                            nc.scalar.copy(out=dd, in_=st[:, :cw])
                        i += 1

            def fold_bias(lnb_c, w_sb, ncols, base_b, bias_dram, dst_sb,
                          ps_pool, rot_pool):
                """Per 512-chunk: row_c = lnb @ W_c + base_c -> DRAM ->
                reload as per-partition bias columns of dst_sb."""
                row = rot_pool.tile([1, ncols], BF16, tag="brow", bufs=1)
                base = rot_pool.tile([1, ncols], BF16, tag="bbase", bufs=1)
                nc.gpsimd.dma_start(
                    out=base,
                    in_=base_b.rearrange("(one n) -> one n", one=1))
                for c0, cw in _chunks(ncols, 512):
                    ps = ps_pool.tile([1, 512], F32, tag="mm")
                    for k in range(KD):
                        nc.tensor.matmul(ps[:, :cw], lnb_c[:, k:k + 1],
                                         w_sb[:, k, c0:c0 + cw],
                                         start=(k == 0), stop=(k == KD - 1))
                    nc.vector.tensor_add(out=row[:, c0:c0 + cw],
                                         in0=ps[:, :cw],
                                         in1=base[:, c0:c0 + cw])
                    nc.sync.dma_start(
                        out=bias_dram.ap()[c0:c0 + cw].rearrange(
                            "(one n) -> one n", one=1),
                        in_=row[:, c0:c0 + cw])
                    nc.gpsimd.dma_start(
                        out=dst_sb[:, c0 // 128:(c0 + cw) // 128],
                        in_=bias_dram.ap()[c0:c0 + cw].rearrange(
                            "(n p) -> p n", p=128))

            def ln_stats(srcb, mu, rstd, chunks, ps_pool, rot_pool):
                """srcb: [128, KD, ncols] bf16 feature-major; mu/rstd: bf16
                [1, ncols] AP slices. rstd = exp(-0.5*ln(var+eps))."""
                ncols = sum(c[1] for c in chunks)
                muf = rot_pool.tile([1, 577], F32, tag="muf", bufs=1)
                msq = rot_pool.tile([1, 577], F32, tag="msq", bufs=1)
                for t0, cnt in chunks:
                    sq = rot_pool.tile([128, KD, 512], BF16, tag="sq", bufs=1)
                    nc.vector.tensor_mul(out=sq[:, :, :cnt],
                                         in0=srcb[:, :, t0:t0 + cnt],
                                         in1=srcb[:, :, t0:t0 + cnt])
                    ps_s = ps_pool.tile([1, 512], F32, tag="mm")
                    for k in range(KD):
                        nc.tensor.matmul(ps_s[:, :cnt], ones_b,
                                         srcb[:, k, t0:t0 + cnt],
                                         start=(k == 0), stop=(k == KD - 1))
                    nc.scalar.mul(out=muf[:, t0:t0 + cnt], in_=ps_s[:, :cnt],
                                  mul=1.0 / D)
                    ps_q = ps_pool.tile([1, 512], F32, tag="mm")
                    for k in range(KD):
                        nc.tensor.matmul(ps_q[:, :cnt], ones_b,
                                         sq[:, k, :cnt],
                                         start=(k == 0), stop=(k == KD - 1))
                    nc.scalar.mul(out=msq[:, t0:t0 + cnt], in_=ps_q[:, :cnt],
                                  mul=1.0 / D)
                var = rot_pool.tile([1, 577], F32, tag="var", bufs=1)
                nc.vector.tensor_mul(out=var[:, :ncols], in0=muf[:, :ncols],
                                     in1=muf[:, :ncols])
                nc.vector.tensor_sub(out=var[:, :ncols], in0=msq[:, :ncols],
                                     in1=var[:, :ncols])
                nc.vector.tensor_copy(out=mu, in_=muf[:, :ncols])
                nc.scalar.activation(out=var[:, :ncols], in_=var[:, :ncols],
                                     func=AF.Ln, bias=eps_t)
                nc.scalar.activation(out=rstd, in_=var[:, :ncols],
                                     func=AF.Exp, scale=-0.5)

            def ln_bcast(mu, rstd, t0, cnt, ps_pool, rot_pool):
                """Broadcast bf16 mu/rstd rows to [128, cnt] bf16."""
                ps_mu = ps_pool.tile([128, 512], F32, tag="mm")
                nc.tensor.matmul(ps_mu[:, :cnt], ones_r, mu[:, t0:t0 + cnt],
                                 start=True, stop=True)
                mu_b = rot_pool.tile([128, 577], BF16, tag="mu_b", bufs=1)
                nc.vector.tensor_copy(out=mu_b[:, :cnt], in_=ps_mu[:, :cnt])
                ps_rs = ps_pool.tile([128, 512], F32, tag="mm")
                nc.tensor.matmul(ps_rs[:, :cnt], ones_r, rstd[:, t0:t0 + cnt],
                                 start=True, stop=True)
                rs_b = rot_pool.tile([128, 577], BF16, tag="rs_b", bufs=1)
                nc.scalar.copy(out=rs_b[:, :cnt], in_=ps_rs[:, :cnt])
                return mu_b, rs_b

            def ln_apply(src_ap, mu_b, rs_b, dst_ap, cnt, rot_pool):
                """dst = (src - mu)*rstd, batched over k (broadcast tiles).
                LN scale/shift live in the folded weights/biases."""
                t1 = rot_pool.tile([128, KD, 577], BF16, tag="lnt1", bufs=1)
                a0, a1 = broadcast_tensor_aps(src_ap, mu_b[:, None, :cnt])
                nc.vector.tensor_sub(out=t1[:, :, :cnt], in0=a0, in1=a1)
                b0, b1 = broadcast_tensor_aps(t1[:, :, :cnt],
                                              rs_b[:, None, :cnt])
                nc.vector.tensor_mul(out=dst_ap, in0=b0, in1=b1)

            # ============ PHASE A ============
            with tc.tile_pool(name="pw", bufs=1) as pw, \
                 tc.tile_pool(name="pa", bufs=2) as pa, \
                 tc.tile_pool(name="pat", bufs=1) as pat, \
                 tc.tile_pool(name="rot", bufs=2) as rot, \
                 tc.tile_pool(name="stg", bufs=2) as stg, \
                 tc.tile_pool(name="ps_wa", bufs=3, space="PSUM") as ps_wa, \
                 tc.tile_pool(name="ps_dp", bufs=3, space="PSUM") as ps_dp, \
                 tc.tile_pool(name="ps_mm", bufs=2, space="PSUM") as ps_mm:

                qkvw = pw.tile([128, KD, 3 * D], BF16, tag="qkvw")
                projw = pw.tile([128, KD, D], BF16, tag="projw")
                def loadx(b):
                    """Load + transpose x(b) to feature-major bf16."""
                    xTb = pa.tile([128, KD, T], BF16, tag="xTb")
                    for tt, (t0, cnt) in enumerate(CH_K):
                        tmb = stg.tile([128, D], BF16, tag="xstageb")
                        nc.gpsimd.dma_start(
                            out=tmb[:cnt, :],
                            in_=x_flat[b * T + t0: b * T + t0 + cnt, :])
                        for k in range(KD):
                            ps_tr = ps_mm.tile([128, 512], BF16, tag="mm")
                            nc.tensor.transpose(ps_tr[:, :cnt],
                                                tmb[:cnt, k * 128:(k + 1) * 128],
                                                ident[:cnt, :cnt])
                            if k % 2 == 0:
                                nc.scalar.copy(out=xTb[:, k, t0:t0 + cnt],
                                               in_=ps_tr[:, :cnt])
                            else:
                                nc.vector.tensor_copy(out=xTb[:, k, t0:t0 + cnt],
                                                      in_=ps_tr[:, :cnt])
                    return xTb

                # load + transpose x(0) before the weight-cast DMA burst
                xTb0 = loadx(0)
                cast_weights(qkv_w.ap(), KD, 3 * D, qkvw, stg, lnw=ln1w_sb)
                cast_weights(proj_w.ap(), KD, D, projw, stg, eng_flip=1)

                # folded qkv bias: qb2 = ln1_b @ (diag(ln1_w) qkv_w) + qkv_b
                qkvb_sb = pp.tile([128, 18], F32, tag="qkvb2")
                fold_bias(ln1b_c, qkvw, 3 * D, qkv_b.ap(), qb2_dram,
                          qkvb_sb, ps_mm, rot)
                vb_bc = pp.tile([128, D], BF16)
                nc.sync.dma_start(
                    out=vb_bc,
                    in_=qb2_dram.ap()[2 * D:3 * D]
                    .rearrange("(one d) -> one d", one=1).to_broadcast((128, D)))

                def front(b, xTb=None):
                    """LN1 + qk/v projections (x already feature-major)."""
                    if xTb is None:
                        xTb = loadx(b)

                    # ---- LN1 -> yT bf16 (scale/shift folded into qkv) ----
                    mu = rot.tile([1, T], BF16, tag="mu", bufs=1)
                    rstd = rot.tile([1, T], BF16, tag="rstd", bufs=1)
                    ln_stats(xTb, mu[:, :], rstd[:, :], CH_T, ps_mm, rot)
                    yT = pa.tile([128, KD, T], BF16, tag="yT", bufs=1)
                    for t0, cnt in CH_T:
                        mu_b, rs_b = ln_bcast(mu, rstd, t0, cnt, ps_mm, rot)
                        ln_apply(xTb[:, :, t0:t0 + cnt], mu_b, rs_b,
                                 yT[:, :, t0:t0 + cnt], cnt, rot)

                    # ---- q,k (feature-major) ----
                    qkT = pa.tile([128, NH, T], BF16, tag="qkT")
                    for n in range(NH):
                        for t0, cnt in CH_T:
                            ps = ps_mm.tile([128, 512], F32, tag="mm")
                            for k in range(KD):
                                nc.tensor.matmul(ps[:, :cnt],
                                                 qkvw[:, k, n * 128:(n + 1) * 128],
                                                 yT[:, k, t0:t0 + cnt],
                                                 start=(k == 0), stop=(k == KD - 1))
                            if n % 2 == 0:
                                nc.scalar.activation(out=qkT[:, n, t0:t0 + cnt],
                                                     in_=ps[:, :cnt],
                                                     func=AF.Identity,
                                                     bias=qkvb_sb[:, n:n + 1])
                            else:
                                nc.vector.tensor_scalar_add(
                                    out=qkT[:, n, t0:t0 + cnt], in0=ps[:, :cnt],
                                    scalar1=qkvb_sb[:, n:n + 1])

                    # ---- v (token-major) ----
                    v_sb = pa.tile([128, len(CH_K), D], BF16, tag="v")
                    for tt, (t0, cnt) in enumerate(CH_K):
                        for f0, fw in ((0, 384), (384, 384)):
                            ps = ps_mm.tile([128, 512], F32, tag="mm")
                            for k in range(KD):
                                nc.tensor.matmul(ps[:cnt, :fw],
                                                 yT[:, k, t0:t0 + cnt],
                                                 qkvw[:, k, 2 * D + f0:2 * D + f0 + fw],
                                                 start=(k == 0), stop=(k == KD - 1))
                            nc.vector.tensor_add(out=v_sb[:cnt, tt, f0:f0 + fw],
                                                 in0=ps[:cnt, :fw],
                                                 in1=vb_bc[:cnt, f0:f0 + fw])
                    return xTb, qkT, v_sb

                def attention(b, xTb, qkT, v_sb):
                    """Softmax-over-heads attention + proj + residual; LN2
                    stats; stash x2 (bf16) to DRAM."""
                    xT2 = pa.tile([128, KD, T], BF16, tag="xT2", bufs=1)
                    for q0, qcnt in CH_Q:
                        wapA = [ps_wa.tile([128, 289], F32, tag="wa",
                                           name=f"wapA{b}_{q0}_{j}")
                                for j in range(3)]
                        attn_p = pat.tile([128, len(CH_K), 6, 289], BF16,
                                          tag="attnp")
                        for kt, (k0, kcnt) in enumerate(CH_K):
                            e_t = rot.tile([128, NH, 289], BF16, tag="e",
                                           bufs=2)
                            for h in range(NH):
                                pb = (h % 2) * 64
                                ps1 = ps_dp.tile([128, 512], F32, tag="dp")
                                nc.tensor.matmul(
                                    ps1[:kcnt, :qcnt],
                                    qkT[pb:pb + 64, 6 + h // 2, k0:k0 + kcnt],
                                    qkT[pb:pb + 64, h // 2, q0:q0 + qcnt],
                                    start=True, stop=True,
                                    tile_position=(pb, 0))
                                nc.scalar.activation(
                                    out=e_t[:kcnt, h, :qcnt],
                                    in_=ps1[:kcnt, :qcnt],
                                    func=AF.Exp, scale=SCALE)
                            # ---- batched tree-sum over heads ----
                            zp6 = rot.tile([128, 6, 289], BF16, tag="zp6",
                                           bufs=1)
                            nc.vector.tensor_add(out=zp6[:kcnt, :, :qcnt],
                                                 in0=e_t[:kcnt, 0:6, :qcnt],
                                                 in1=e_t[:kcnt, 6:12, :qcnt])
                            z3 = rot.tile([128, 3, 289], BF16, tag="z3",
                                          bufs=1)
                            nc.vector.tensor_add(out=z3[:kcnt, :, :qcnt],
                                                 in0=zp6[:kcnt, 0:3, :qcnt],
                                                 in1=zp6[:kcnt, 3:6, :qcnt])
                            za = rot.tile([128, 289], BF16, tag="za", bufs=1)
                            nc.vector.tensor_add(out=za[:kcnt, :qcnt],
                                                 in0=z3[:kcnt, 0, :qcnt],
                                                 in1=z3[:kcnt, 1, :qcnt])
                            z = rot.tile([128, 289], F32, tag="z", bufs=1)
                            nc.vector.tensor_add(out=z[:kcnt, :qcnt],
                                                 in0=za[:kcnt, :qcnt],
                                                 in1=z3[:kcnt, 2, :qcnt])
                            rz = rot.tile([128, 289], F32, tag="rz", bufs=1)
                            nc.vector.reciprocal_approx_fast(
                                out=rz[:kcnt, :qcnt], in_=z[:kcnt, :qcnt])
                            rzb = rot.tile([128, 289], BF16, tag="rzb", bufs=1)
                            nc.vector.tensor_copy(out=rzb[:kcnt, :qcnt],
                                                  in_=rz[:kcnt, :qcnt])
                            # ---- normalize (broadcast over heads) ----
                            attn = rot.tile([128, 6, 289], BF16, tag="attn",
                                            bufs=1)
                            a0, a1 = broadcast_tensor_aps(
                                e_t[:kcnt, 0:6, :qcnt], rzb[:kcnt, None, :qcnt])
                            nc.vector.tensor_mul(out=attn[:kcnt, :, :qcnt],
                                                 in0=a0, in1=a1)
                            b0, b1 = broadcast_tensor_aps(
                                e_t[:kcnt, 6:12, :qcnt], rzb[:kcnt, None, :qcnt])
                            nc.vector.tensor_mul(
                                out=attn_p[:kcnt, kt, :, :qcnt], in0=b0, in1=b1)
                            for h in range(6):
                                pb = (h % 2) * 64
                                nc.tensor.matmul(
                                    wapA[h // 2][pb:pb + 64, :qcnt],
                                    v_sb[:kcnt, kt, h * 64:(h + 1) * 64],
                                    attn[:kcnt, h, :qcnt],
                                    start=(kt == 0), stop=(kt == len(CH_K) - 1),
                                    tile_position=(0, pb),
                                    skip_group_check=True)
                        waB = pat.tile([128, KD, 289], BF16, tag="waB")
                        for j in range(3):
                            nc.vector.tensor_copy(out=waB[:, j, :qcnt],
                                                  in_=wapA[j][:, :qcnt])
                        wapB = [ps_wa.tile([128, 289], F32, tag="wa",
                                           name=f"wapB{b}_{q0}_{j}")
                                for j in range(3)]
                        for kt, (k0, kcnt) in enumerate(CH_K):
                            for h in range(6, NH):
                                pb = (h % 2) * 64
                                nc.tensor.matmul(
                                    wapB[(h - 6) // 2][pb:pb + 64, :qcnt],
                                    v_sb[:kcnt, kt, h * 64:(h + 1) * 64],
                                    attn_p[:kcnt, kt, h - 6, :qcnt],
                                    start=(kt == 0), stop=(kt == len(CH_K) - 1),
                                    tile_position=(0, pb),
                                    skip_group_check=True)
                        for j in range(3):
                            if j % 2 == 0:
                                nc.vector.tensor_copy(out=waB[:, 3 + j, :qcnt],
                                                      in_=wapB[j][:, :qcnt])
                            else:
                                nc.scalar.copy(out=waB[:, 3 + j, :qcnt],
                                               in_=wapB[j][:, :qcnt])

                        # ---- proj + residual for this q-chunk ----
                        for n in range(KD):
                            ps = ps_mm.tile([128, 512], F32, tag="mm")
                            for k in range(KD):
                                nc.tensor.matmul(ps[:, :qcnt],
                                                 projw[:, k, n * 128:(n + 1) * 128],
                                                 waB[:, k, :qcnt],
                                                 start=(k == 0), stop=(k == KD - 1))
                            nc.vector.scalar_tensor_tensor(
                                out=xT2[:, n, q0:q0 + qcnt], in0=ps[:, :qcnt],
                                scalar=projb_sb[:, n:n + 1],
                                in1=xTb[:, n, q0:q0 + qcnt],
                                op0=ALU.add, op1=ALU.add)

                    # ---- LN2 stats for this batch + stash ----
                    ln_stats(xT2, mu2[:, b * T:(b + 1) * T],
                             rstd2[:, b * T:(b + 1) * T], CH_T, ps_mm, rot)
                    for k in range(KD):
                        nc.sync.dma_start(out=x2_dram.ap()[k, :, b * T:(b + 1) * T],
                                          in_=xT2[:, k, :])

                def prep_fcw(dram_src, dram_dst, ktiles, ncols):
                    """f32 DRAM -> (SWDGE cast) -> bf16 SBUF -> bf16 DRAM.
                    Pure DMA; uses idle phase-A DMA bandwidth so phase B
                    loads bf16 directly."""
                    for k in range(ktiles):
                        for c0, cw in _chunks(ncols, 512):
                            stp = stg.tile([128, 512], BF16, tag="fcprep",
                                           bufs=2)
                            nc.gpsimd.dma_start(
                                out=stp[:, :cw],
                                in_=dram_src[k * 128:(k + 1) * 128,
                                             c0:c0 + cw])
                            nc.sync.dma_start(
                                out=dram_dst.ap()[k, :, c0:c0 + cw],
                                in_=stp[:, :cw])

                state = front(0, xTb0)
                for b in range(BL):
                    nxt = front(b + 1) if b + 1 < BL else None
                    if b == 1:
                        prep_fcw(fc1_w.ap(), fc1w_dram, KD, HID)
                    if b == 2:
                        prep_fcw(fc2_w.ap(), fc2w_dram, KH, D)
                    attention(b, *state)
                    state = nxt

            # ============ PHASE B (MLP over global tokens) ============
            with tc.tile_pool(name="pwB", bufs=1) as pwB, \
                 tc.tile_pool(name="pb", bufs=2) as pb, \
                 tc.tile_pool(name="pbh", bufs=1) as pbh, \
                 tc.tile_pool(name="stgB", bufs=2) as stgB, \
                 tc.tile_pool(name="ps_mmB", bufs=3, space="PSUM") as ps_mmB, \
                 tc.tile_pool(name="ps_trB", bufs=2, space="PSUM") as ps_trB:

                def load_bf16(dram, ktiles, ncols, dst):
                    for c0, cw in _chunks(ncols, 512):
                        for k in range(ktiles):
                            nc.sync.dma_start(
                                out=dst[:, k, c0:c0 + cw],
                                in_=dram.ap()[k, :, c0:c0 + cw])

                fc1w = pwB.tile([128, KD, HID], BF16, tag="fc1w")
                load_bf16(fc1w_dram, KD, HID, fc1w)
                fc1b_sb = pp.tile([128, 24], F32, tag="fc1b2")
                fold_bias(ln2b_c, fc1w, HID, fc1_b.ap(), fb2_dram,
                          fc1b_sb, ps_mmB, pb)
                fc2w = pwB.tile([128, KH, D], BF16, tag="fc2w")
                load_bf16(fc2w_dram, KH, D, fc2w)

                for g0, cnt in CH_G:
                    x2c = pb.tile([128, KD, 512], BF16, tag="x2c")
                    for k in range(KD):
                        nc.sync.dma_start(out=x2c[:, k, :cnt],
                                          in_=x2_dram.ap()[k, :, g0:g0 + cnt])
                    mu_b, rs_b = ln_bcast(mu2, rstd2, g0, cnt, ps_mmB, pb)
                    zT = pb.tile([128, KD, 512], BF16, tag="zT")
                    ln_apply(x2c[:, :, :cnt], mu_b, rs_b, zT[:, :, :cnt],
                             cnt, pb)
                    for k in range(KD):
                        nc.vector.tensor_scalar_mul(
                            out=zT[:, k, :cnt], in0=zT[:, k, :cnt],
                            scalar1=ln2w_sb[:, k:k + 1])

                    h_t = pbh.tile([128, KH, 512], BF16, tag="h")
                    for n in range(KH):
                        ps = ps_mmB.tile([128, 512], F32, tag="mmB")
                        for k in range(KD):
                            nc.tensor.matmul(ps[:, :cnt],
                                             fc1w[:, k, n * 128:(n + 1) * 128],
                                             zT[:, k, :cnt],
                                             start=(k == 0), stop=(k == KD - 1))
                        nc.scalar.activation(out=h_t[:, n, :cnt], in_=ps[:, :cnt],
                                             func=AF.Gelu,
                                             bias=fc1b_sb[:, n:n + 1])
                    xf = pb.tile([128, KD, 512], BF16, tag="xf")
                    for n in range(KD):
                        ps = ps_mmB.tile([128, 512], F32, tag="mmB")
                        for k in range(KH):
                            nc.tensor.matmul(ps[:, :cnt],
                                             fc2w[:, k, n * 128:(n + 1) * 128],
                                             h_t[:, k, :cnt],
                                             start=(k == 0), stop=(k == KH - 1))
                        nc.vector.scalar_tensor_tensor(
                            out=xf[:, n, :cnt], in0=ps[:, :cnt],
                            scalar=fc2b_sb[:, n:n + 1],
                            in1=x2c[:, n, :cnt],
                            op0=ALU.add, op1=ALU.add)
                    # ---- transpose back to token-major and store ----
                    for c0, ccnt in _chunks(cnt, 128):
                        om = stgB.tile([128, D], BF16, tag="om")
                        for k in range(KD):
                            ps_tr = ps_trB.tile([128, 128], BF16, tag="tr")
                            nc.tensor.transpose(ps_tr[:ccnt, :],
                                                xf[:, k, c0:c0 + ccnt], ident)
                            if k % 2 == 0:
                                nc.scalar.copy(out=om[:ccnt, k * 128:(k + 1) * 128],
                                               in_=ps_tr[:ccnt, :])
                            else:
                                nc.vector.tensor_copy(
                                    out=om[:ccnt, k * 128:(k + 1) * 128],
                                    in_=ps_tr[:ccnt, :])
                        nc.gpsimd.dma_start(
                            out=out_flat[g0 + c0:g0 + c0 + ccnt, :],
                            in_=om[:ccnt, :])

    nc.compile()
    return nc


def kernel(**inputs) -> np.ndarray:
    if "nc" in _NC_CACHE:
        nc = _NC_CACHE["nc"]
    else:
        nc = _NC_CACHE["nc"] = build_nc()
    x = np.ascontiguousarray(np.asarray(inputs["x"], dtype=np.float32))
    weights = {k: np.ascontiguousarray(np.asarray(v, dtype=np.float32))
               for k, v in inputs.items() if k != "x"}
    in_maps = []
    for c in range(N_CORES):
        m = {"x": x[c * BL:(c + 1) * BL]}
        m.update(weights)
        in_maps.append(m)
    last_err = None
    for attempt in range(3):
        try:
            r = run_bass_kernel_spmd(nc, in_maps, core_ids=list(range(N_CORES)))
            return np.concatenate([r.results[c]["out"] for c in range(N_CORES)],
                                  axis=0)
        except Exception as e:  # transient device flakes: retry
            last_err = e
    raise last_err


# revision 15
# speedup vs baseline: 70.2355x; 1.0349x over previous
"""Trainium2 Bass kernel for a pre-LN transformer block (dense_transformer).

Problem shapes (hardcoded): x [32, 577, 768], 12 heads, dh=64, mlp 3072.
NOTE: softmax in the reference is over the HEADS axis (dim=1 of [B,h,T,T]),
replicated faithfully here.

Sharding: pure data-parallel over batch: 8 cores x 4 batches each.
Weights replicated. No collectives.

Layout: activations feature-major ([feature partitions, tokens free]) so
matmuls consume natural weight tiles [k,n]; matmuls in bf16 with fp32 PSUM.

Structure for PE density (HAM warmth): phase A emits batch b+1's front
(load/LN1/qkv/v) before batch b's attention so the scheduler backfills the
PE during softmax stretches; proj runs per q-chunk. Phase B (MLP) uses
LN2 stats precomputed in phase A and double-buffered token chunks.

Softmax-over-heads: paired-PSUM-bank exp, batched multi-dim DVE tree-sum,
reciprocal_approx_fast, head-broadcast normalize.

LN scale/shift are folded into the following matmul: W' = diag(w) @ W at
weight-cast time and b' = b_ln @ W + b at kernel start (tiny PE matmuls +
a DRAM roundtrip to relayout the folded bias row per-partition).
"""
import numpy as np

import concourse.bacc as bacc
import concourse.mybir as mybir
import concourse.tile as tile
from concourse.bass import broadcast_tensor_aps
from concourse.bass_utils import run_bass_kernel_spmd
from concourse.masks import make_identity

F32 = mybir.dt.float32
BF16 = mybir.dt.bfloat16
AF = mybir.ActivationFunctionType
ALU = mybir.AluOpType

N_CORES = 8
B, T, D = 32, 577, 768
BL = B // N_CORES          # 4 batches per core
NH, DH = 12, 64            # heads
HID = 4 * D                # 3072
KD = D // 128              # 6 feature tiles
KH = HID // 128            # 24 hidden tiles
EPS = 1e-6
SCALE = DH ** -0.5

TOK = BL * T               # 2308 tokens per core
CH_T = [(0, 512), (512, 65)]                      # within one batch (577)
CH_Q = [(0, 289), (289, 288)]                     # attention q chunks
CH_K = [(0, 128), (128, 128), (256, 128), (384, 128), (512, 65)]  # kt tiles
CH_G = [(0, 512), (512, 512), (1024, 512), (1536, 512), (2048, 260)]  # global

_NC_CACHE = {}


def _chunks(total, step):
    out = []
    o = 0
    while o < total:
        out.append((o, min(step, total - o)))
        o += step
    return out


def build_nc():
    nc = bacc.Bacc("TRN2")
    x = nc.dram_tensor("x", [BL, T, D], F32, kind="ExternalInput")
    ln1_w = nc.dram_tensor("ln1_w", [D], F32, kind="ExternalInput")
    ln1_b = nc.dram_tensor("ln1_b", [D], F32, kind="ExternalInput")
    qkv_w = nc.dram_tensor("qkv_w", [D, 3 * D], F32, kind="ExternalInput")
    qkv_b = nc.dram_tensor("qkv_b", [3 * D], F32, kind="ExternalInput")
    proj_w = nc.dram_tensor("proj_w", [D, D], F32, kind="ExternalInput")
    proj_b = nc.dram_tensor("proj_b", [D], F32, kind="ExternalInput")
    ln2_w = nc.dram_tensor("ln2_w", [D], F32, kind="ExternalInput")
    ln2_b = nc.dram_tensor("ln2_b", [D], F32, kind="ExternalInput")
    fc1_w = nc.dram_tensor("fc1_w", [D, HID], F32, kind="ExternalInput")
    fc1_b = nc.dram_tensor("fc1_b", [HID], F32, kind="ExternalInput")
    fc2_w = nc.dram_tensor("fc2_w", [HID, D], F32, kind="ExternalInput")
    fc2_b = nc.dram_tensor("fc2_b", [D], F32, kind="ExternalInput")
    out = nc.dram_tensor("out", [BL, T, D], F32, kind="ExternalOutput")

    # DRAM scratch
    x2_dram = nc.dram_tensor("x2_dram", [KD, 128, TOK], BF16, kind="Internal")
    fc1w_dram = nc.dram_tensor("fc1w_dram", [KD, 128, HID], BF16, kind="Internal")
    fc2w_dram = nc.dram_tensor("fc2w_dram", [KH, 128, D], BF16, kind="Internal")
    qb2_dram = nc.dram_tensor("qb2_dram", [3 * D], BF16, kind="Internal")
    fb2_dram = nc.dram_tensor("fb2_dram", [HID], BF16, kind="Internal")

    x_flat = x.ap().rearrange("b t d -> (b t) d")      # [2308, 768]
    out_flat = out.ap().rearrange("b t d -> (b t) d")

    with tile.TileContext(nc) as tc:
        with tc.tile_pool(name="persist", bufs=1) as pp:
            ident = pp.tile([128, 128], BF16)
            make_identity(nc, ident)
            ones_b = pp.tile([128, 1], BF16)
            nc.vector.memset(ones_b, 1.0)
            ones_r = pp.tile([1, 128], BF16)   # broadcast lhsT (partition 0)
            nc.vector.memset(ones_r, 1.0)
            eps_t = pp.tile([1, 1], F32)
            nc.vector.memset(eps_t, EPS)

            def load_cols(name, dram_ap, n, pool=None, cast=False):
                t = (pool or pp).tile([128, n], F32, tag=name)
                eng = nc.gpsimd if cast else nc.sync
                eng.dma_start(
                    out=t, in_=dram_ap.rearrange("(n p) -> p n", p=128))
                return t

            projb_sb = load_cols("projb", proj_b.ap(), 6)
            fc2b_sb = load_cols("fc2b", fc2_b.ap(), 6)
            ln1w_sb = load_cols("ln1w", ln1_w.ap(), 6)
            ln2w_sb = load_cols("ln2w", ln2_w.ap(), 6)
            ln1b_sb = load_cols("ln1b", ln1_b.ap(), 6)
            ln2b_sb = load_cols("ln2b", ln2_b.ap(), 6)
            ln1b_c = pp.tile([128, KD], BF16)
            nc.vector.tensor_copy(out=ln1b_c, in_=ln1b_sb)
            ln2b_c = pp.tile([128, KD], BF16)
            nc.vector.tensor_copy(out=ln2b_c, in_=ln2b_sb)
            # LN2 stats computed in phase A, consumed in phase B
            mu2 = pp.tile([1, TOK], BF16, tag="mu2")
            rstd2 = pp.tile([1, TOK], BF16, tag="rstd2")

            # ---------------- shared helpers ----------------
            def cast_weights(dram, ktiles, ncols, dst, stg, lnw=None,
                             eng_flip=0):
                """Cast f32 weights to resident bf16, n-major (chunk-outer)
                so consumers of early n-tiles start before the full load;
                optionally fold the preceding layernorm's scale:
                W'[p,:] = lnw[p] * W[p,:]."""
                i = 0
                for c0, cw in _chunks(ncols, 512):
                    for k in range(ktiles):
                        st = stg.tile([128, 512], F32, tag="wstage", bufs=2)
                        nc.sync.dma_start(
                            out=st[:, :cw],
                            in_=dram[k * 128:(k + 1) * 128, c0:c0 + cw])
                        dd = dst[:, k, c0:c0 + cw]
                        if lnw is not None:
                            nc.vector.tensor_scalar_mul(
                                out=dd, in0=st[:, :cw],
                                scalar1=lnw[:, k:k + 1])
                        elif (i + eng_flip) % 2 == 0:
                            nc.vector.tensor_copy(out=dd, in_=st[:, :cw])
                        else:
                            nc.scalar.copy(out=dd, in_=st[:, :cw])
                        i += 1

            def fold_bias(lnb_c, w_sb, ncols, base_b, bias_dram, dst_sb,
                          ps_pool, rot_pool):
                """Per 512-chunk: row_c = lnb @ W_c + base_c -> DRAM ->
                reload as per-partition bias columns of dst_sb."""
                row = rot_pool.tile([1, ncols], BF16, tag="brow", bufs=1)
                base = rot_pool.tile([1, ncols], BF16, tag="bbase", bufs=1)
                nc.gpsimd.dma_start(
                    out=base,
                    in_=base_b.rearrange("(one n) -> one n", one=1))
                for c0, cw in _chunks(ncols, 512):
                    ps = ps_pool.tile([1, 512], F32, tag="mm")
                    for k in range(KD):
                        nc.tensor.matmul(ps[:, :cw], lnb_c[:, k:k + 1],
                                         w_sb[:, k, c0:c0 + cw],
                                         start=(k == 0), stop=(k == KD - 1))
                    nc.vector.tensor_add(out=row[:, c0:c0 + cw],
                                         in0=ps[:, :cw],
                                         in1=base[:, c0:c0 + cw])
                    nc.scalar.dma_start(
                        out=bias_dram.ap()[c0:c0 + cw].rearrange(
                            "(one n) -> one n", one=1),
                        in_=row[:, c0:c0 + cw])
                    nc.gpsimd.dma_start(
                        out=dst_sb[:, c0 // 128:(c0 + cw) // 128],
                        in_=bias_dram.ap()[c0:c0 + cw].rearrange(
                            "(n p) -> p n", p=128))

            def ln_stats(srcb, mu, rstd, chunks, ps_pool, rot_pool):
                """srcb: [128, KD, ncols] bf16 feature-major; mu/rstd: bf16
                [1, ncols] AP slices. rstd = exp(-0.5*ln(var+eps))."""
                ncols = sum(c[1] for c in chunks)
                muf = rot_pool.tile([1, 577], F32, tag="muf", bufs=1)
                msq = rot_pool.tile([1, 577], F32, tag="msq", bufs=1)
                for t0, cnt in chunks:
                    sq = rot_pool.tile([128, KD, 512], BF16, tag="sq", bufs=1)
                    nc.gpsimd.tensor_mul(out=sq[:, :, :cnt],
                                         in0=srcb[:, :, t0:t0 + cnt],
                                         in1=srcb[:, :, t0:t0 + cnt])
                    ps_s = ps_pool.tile([1, 512], F32, tag="mm")
                    for k in range(KD):
                        nc.tensor.matmul(ps_s[:, :cnt], ones_b,
                                         srcb[:, k, t0:t0 + cnt],
                                         start=(k == 0), stop=(k == KD - 1))
                    nc.scalar.mul(out=muf[:, t0:t0 + cnt], in_=ps_s[:, :cnt],
                                  mul=1.0 / D)
                    ps_q = ps_pool.tile([1, 512], F32, tag="mm")
                    for k in range(KD):
                        nc.tensor.matmul(ps_q[:, :cnt], ones_b,
                                         sq[:, k, :cnt],
                                         start=(k == 0), stop=(k == KD - 1))
                    nc.scalar.mul(out=msq[:, t0:t0 + cnt], in_=ps_q[:, :cnt],
                                  mul=1.0 / D)
                var = rot_pool.tile([1, 577], F32, tag="var", bufs=1)
                nc.vector.tensor_mul(out=var[:, :ncols], in0=muf[:, :ncols],
                                     in1=muf[:, :ncols])
                nc.vector.tensor_sub(out=var[:, :ncols], in0=msq[:, :ncols],
                                     in1=var[:, :ncols])
                nc.vector.tensor_copy(out=mu, in_=muf[:, :ncols])
                nc.scalar.activation(out=var[:, :ncols], in_=var[:, :ncols],
                                     func=AF.Ln, bias=eps_t)
                nc.scalar.activation(out=rstd, in_=var[:, :ncols],
                                     func=AF.Exp, scale=-0.5)

            def ln_bcast(mu, rstd, t0, cnt, ps_pool, rot_pool):
                """Broadcast bf16 mu/rstd rows to [128, cnt] bf16."""
                ps_mu = ps_pool.tile([128, 512], F32, tag="mm")
                nc.tensor.matmul(ps_mu[:, :cnt], ones_r, mu[:, t0:t0 + cnt],
                                 start=True, stop=True)
                mu_b = rot_pool.tile([128, 577], BF16, tag="mu_b", bufs=1)
                nc.vector.tensor_copy(out=mu_b[:, :cnt], in_=ps_mu[:, :cnt])
                ps_rs = ps_pool.tile([128, 512], F32, tag="mm")
                nc.tensor.matmul(ps_rs[:, :cnt], ones_r, rstd[:, t0:t0 + cnt],
                                 start=True, stop=True)
                rs_b = rot_pool.tile([128, 577], BF16, tag="rs_b", bufs=1)
                nc.scalar.copy(out=rs_b[:, :cnt], in_=ps_rs[:, :cnt])
                return mu_b, rs_b

            def ln_apply(src_ap, mu_b, rs_b, dst_ap, cnt, rot_pool):
                """dst = (src - mu)*rstd, batched over k (broadcast tiles).
                LN scale/shift live in the folded weights/biases."""
                t1 = rot_pool.tile([128, KD, 577], BF16, tag="lnt1", bufs=1)
                a0, a1 = broadcast_tensor_aps(src_ap, mu_b[:, None, :cnt])
                nc.vector.tensor_sub(out=t1[:, :, :cnt], in0=a0, in1=a1)
                b0, b1 = broadcast_tensor_aps(t1[:, :, :cnt],
                                              rs_b[:, None, :cnt])
                nc.vector.tensor_mul(out=dst_ap, in0=b0, in1=b1)

            # ============ PHASE A ============
            with tc.tile_pool(name="pw", bufs=1) as pw, \
                 tc.tile_pool(name="pa", bufs=2) as pa, \
                 tc.tile_pool(name="pat", bufs=1) as pat, \
                 tc.tile_pool(name="rot", bufs=2) as rot, \
                 tc.tile_pool(name="stg", bufs=2) as stg, \
                 tc.tile_pool(name="ps_wa", bufs=3, space="PSUM") as ps_wa, \
                 tc.tile_pool(name="ps_dp", bufs=3, space="PSUM") as ps_dp, \
                 tc.tile_pool(name="ps_mm", bufs=2, space="PSUM") as ps_mm:

                qkvw = pw.tile([128, KD, 3 * D], BF16, tag="qkvw")
                projw = pw.tile([128, KD, D], BF16, tag="projw")
                def loadx(b):
                    """Load + transpose x(b) to feature-major bf16."""
                    xTb = pa.tile([128, KD, T], BF16, tag="xTb")
                    for tt, (t0, cnt) in enumerate(CH_K):
                        tmb = stg.tile([128, D], BF16, tag="xstageb")
                        nc.gpsimd.dma_start(
                            out=tmb[:cnt, :],
                            in_=x_flat[b * T + t0: b * T + t0 + cnt, :])
                        for k in range(KD):
                            ps_tr = ps_mm.tile([128, 512], BF16, tag="mm")
                            nc.tensor.transpose(ps_tr[:, :cnt],
                                                tmb[:cnt, k * 128:(k + 1) * 128],
                                                ident[:cnt, :cnt])
                            if k % 2 == 0:
                                nc.scalar.copy(out=xTb[:, k, t0:t0 + cnt],
                                               in_=ps_tr[:, :cnt])
                            else:
                                nc.vector.tensor_copy(out=xTb[:, k, t0:t0 + cnt],
                                                      in_=ps_tr[:, :cnt])
                    return xTb

                # load + transpose x(0) before the weight-cast DMA burst
                xTb0 = loadx(0)
                cast_weights(qkv_w.ap(), KD, 3 * D, qkvw, stg, lnw=ln1w_sb)
                cast_weights(proj_w.ap(), KD, D, projw, stg, eng_flip=1)

                # folded qkv bias: qb2 = ln1_b @ (diag(ln1_w) qkv_w) + qkv_b
                qkvb_sb = pp.tile([128, 18], F32, tag="qkvb2")
                fold_bias(ln1b_c, qkvw, 3 * D, qkv_b.ap(), qb2_dram,
                          qkvb_sb, ps_mm, rot)
                vb_bc = pp.tile([128, D], BF16)
                nc.scalar.dma_start(
                    out=vb_bc,
                    in_=qb2_dram.ap()[2 * D:3 * D]
                    .rearrange("(one d) -> one d", one=1).to_broadcast((128, D)))

                def front(b, xTb=None):
                    """LN1 + qk/v projections (x already feature-major)."""
                    if xTb is None:
                        xTb = loadx(b)

                    # ---- LN1 -> yT bf16 (scale/shift folded into qkv) ----
                    mu = rot.tile([1, T], BF16, tag="mu", bufs=1)
                    rstd = rot.tile([1, T], BF16, tag="rstd", bufs=1)
                    ln_stats(xTb, mu[:, :], rstd[:, :], CH_T, ps_mm, rot)
                    yT = pa.tile([128, KD, T], BF16, tag="yT", bufs=1)
                    for t0, cnt in CH_T:
                        mu_b, rs_b = ln_bcast(mu, rstd, t0, cnt, ps_mm, rot)
                        ln_apply(xTb[:, :, t0:t0 + cnt], mu_b, rs_b,
                                 yT[:, :, t0:t0 + cnt], cnt, rot)

                    # ---- q,k (feature-major) ----
                    qkT = pa.tile([128, NH, T], BF16, tag="qkT")
                    for n in range(NH):
                        for t0, cnt in CH_T:
                            ps = ps_mm.tile([128, 512], F32, tag="mm")
                            for k in range(KD):
                                nc.tensor.matmul(ps[:, :cnt],
                                                 qkvw[:, k, n * 128:(n + 1) * 128],
                                                 yT[:, k, t0:t0 + cnt],
                                                 start=(k == 0), stop=(k == KD - 1))
                            if n % 2 == 0:
                                nc.scalar.activation(out=qkT[:, n, t0:t0 + cnt],
                                                     in_=ps[:, :cnt],
                                                     func=AF.Identity,
                                                     bias=qkvb_sb[:, n:n + 1])
                            else:
                                nc.vector.tensor_scalar_add(
                                    out=qkT[:, n, t0:t0 + cnt], in0=ps[:, :cnt],
                                    scalar1=qkvb_sb[:, n:n + 1])

                    # ---- v (token-major) ----
                    v_sb = pa.tile([128, len(CH_K), D], BF16, tag="v")
                    for tt, (t0, cnt) in enumerate(CH_K):
                        for f0, fw in ((0, 384), (384, 384)):
                            ps = ps_mm.tile([128, 512], F32, tag="mm")
                            for k in range(KD):
                                nc.tensor.matmul(ps[:cnt, :fw],
                                                 yT[:, k, t0:t0 + cnt],
                                                 qkvw[:, k, 2 * D + f0:2 * D + f0 + fw],
                                                 start=(k == 0), stop=(k == KD - 1))
                            nc.vector.tensor_add(out=v_sb[:cnt, tt, f0:f0 + fw],
                                                 in0=ps[:cnt, :fw],
                                                 in1=vb_bc[:cnt, f0:f0 + fw])
                    return xTb, qkT, v_sb

                def attention(b, xTb, qkT, v_sb):
                    """Softmax-over-heads attention + proj + residual; LN2
                    stats; stash x2 (bf16) to DRAM."""
                    xT2 = pa.tile([128, KD, T], BF16, tag="xT2", bufs=1)
                    for q0, qcnt in CH_Q:
                        wapA = [ps_wa.tile([128, 289], F32, tag="wa",
                                           name=f"wapA{b}_{q0}_{j}")
                                for j in range(3)]
                        attn_p = pat.tile([128, len(CH_K), 6, 289], BF16,
                                          tag="attnp")
                        for kt, (k0, kcnt) in enumerate(CH_K):
                            e_t = rot.tile([128, NH, 289], BF16, tag="e",
                                           bufs=2)
                            for h in range(NH):
                                pb = (h % 2) * 64
                                ps1 = ps_dp.tile([128, 512], F32, tag="dp")
                                nc.tensor.matmul(
                                    ps1[:kcnt, :qcnt],
                                    qkT[pb:pb + 64, 6 + h // 2, k0:k0 + kcnt],
                                    qkT[pb:pb + 64, h // 2, q0:q0 + qcnt],
                                    start=True, stop=True,
                                    tile_position=(pb, 0))
                                nc.scalar.activation(
                                    out=e_t[:kcnt, h, :qcnt],
                                    in_=ps1[:kcnt, :qcnt],
                                    func=AF.Exp, scale=SCALE)
                            # ---- batched tree-sum over heads ----
                            zp6 = rot.tile([128, 6, 289], BF16, tag="zp6",
                                           bufs=1)
                            nc.vector.tensor_add(out=zp6[:kcnt, :, :qcnt],
                                                 in0=e_t[:kcnt, 0:6, :qcnt],
                                                 in1=e_t[:kcnt, 6:12, :qcnt])
                            z3 = rot.tile([128, 3, 289], BF16, tag="z3",
                                          bufs=1)
                            nc.vector.tensor_add(out=z3[:kcnt, :, :qcnt],
                                                 in0=zp6[:kcnt, 0:3, :qcnt],
                                                 in1=zp6[:kcnt, 3:6, :qcnt])
                            za = rot.tile([128, 289], BF16, tag="za", bufs=1)
                            nc.vector.tensor_add(out=za[:kcnt, :qcnt],
                                                 in0=z3[:kcnt, 0, :qcnt],
                                                 in1=z3[:kcnt, 1, :qcnt])
                            z = rot.tile([128, 289], F32, tag="z", bufs=1)
                            nc.vector.tensor_add(out=z[:kcnt, :qcnt],
                                                 in0=za[:kcnt, :qcnt],
                                                 in1=z3[:kcnt, 2, :qcnt])
                            rz = rot.tile([128, 289], F32, tag="rz", bufs=1)
                            nc.vector.reciprocal_approx_fast(
                                out=rz[:kcnt, :qcnt], in_=z[:kcnt, :qcnt])
                            rzb = rot.tile([128, 289], BF16, tag="rzb", bufs=1)
                            nc.vector.tensor_copy(out=rzb[:kcnt, :qcnt],
                                                  in_=rz[:kcnt, :qcnt])
                            # ---- normalize (broadcast over heads) ----
                            attn = rot.tile([128, 6, 289], BF16, tag="attn",
                                            bufs=1)
                            a0, a1 = broadcast_tensor_aps(
                                e_t[:kcnt, 0:6, :qcnt], rzb[:kcnt, None, :qcnt])
                            nc.vector.tensor_mul(out=attn[:kcnt, :, :qcnt],
                                                 in0=a0, in1=a1)
                            b0, b1 = broadcast_tensor_aps(
                                e_t[:kcnt, 6:12, :qcnt], rzb[:kcnt, None, :qcnt])
                            nc.vector.tensor_mul(
                                out=attn_p[:kcnt, kt, :, :qcnt], in0=b0, in1=b1)
                            for h in range(6):
                                pb = (h % 2) * 64
                                nc.tensor.matmul(
                                    wapA[h // 2][pb:pb + 64, :qcnt],
                                    v_sb[:kcnt, kt, h * 64:(h + 1) * 64],
                                    attn[:kcnt, h, :qcnt],
                                    start=(kt == 0), stop=(kt == len(CH_K) - 1),
                                    tile_position=(0, pb),
                                    skip_group_check=True)
                        waB = pat.tile([128, KD, 289], BF16, tag="waB")
                        for j in range(3):
                            nc.vector.tensor_copy(out=waB[:, j, :qcnt],
                                                  in_=wapA[j][:, :qcnt])
                        wapB = [ps_wa.tile([128, 289], F32, tag="wa",
                                           name=f"wapB{b}_{q0}_{j}")
                                for j in range(3)]
                        for kt, (k0, kcnt) in enumerate(CH_K):
                            for h in range(6, NH):
                                pb = (h % 2) * 64
                                nc.tensor.matmul(
                                    wapB[(h - 6) // 2][pb:pb + 64, :qcnt],
                                    v_sb[:kcnt, kt, h * 64:(h + 1) * 64],
                                    attn_p[:kcnt, kt, h - 6, :qcnt],
                                    start=(kt == 0), stop=(kt == len(CH_K) - 1),
                                    tile_position=(0, pb),
                                    skip_group_check=True)
                        for j in range(3):
                            if j % 2 == 0:
                                nc.vector.tensor_copy(out=waB[:, 3 + j, :qcnt],
                                                      in_=wapB[j][:, :qcnt])
                            else:
                                nc.scalar.copy(out=waB[:, 3 + j, :qcnt],
                                               in_=wapB[j][:, :qcnt])

                        # ---- proj + residual for this q-chunk ----
                        for n in range(KD):
                            ps = ps_mm.tile([128, 512], F32, tag="mm")
                            for k in range(KD):
                                nc.tensor.matmul(ps[:, :qcnt],
                                                 projw[:, k, n * 128:(n + 1) * 128],
                                                 waB[:, k, :qcnt],
                                                 start=(k == 0), stop=(k == KD - 1))
                            nc.vector.scalar_tensor_tensor(
                                out=xT2[:, n, q0:q0 + qcnt], in0=ps[:, :qcnt],
                                scalar=projb_sb[:, n:n + 1],
                                in1=xTb[:, n, q0:q0 + qcnt],
                                op0=ALU.add, op1=ALU.add)

                    # ---- LN2 stats for this batch + stash ----
                    ln_stats(xT2, mu2[:, b * T:(b + 1) * T],
                             rstd2[:, b * T:(b + 1) * T], CH_T, ps_mm, rot)
                    for k in range(KD):
                        nc.sync.dma_start(out=x2_dram.ap()[k, :, b * T:(b + 1) * T],
                                          in_=xT2[:, k, :])

                def prep_fcw(dram_src, dram_dst, ktiles, ncols):
                    """f32 DRAM -> (SWDGE cast) -> bf16 SBUF -> bf16 DRAM.
                    Pure DMA; uses idle phase-A DMA bandwidth so phase B
                    loads bf16 directly."""
                    for k in range(ktiles):
                        for c0, cw in _chunks(ncols, 512):
                            stp = stg.tile([128, 512], BF16, tag="fcprep",
                                           bufs=2)
                            nc.gpsimd.dma_start(
                                out=stp[:, :cw],
                                in_=dram_src[k * 128:(k + 1) * 128,
                                             c0:c0 + cw])
                            nc.sync.dma_start(
                                out=dram_dst.ap()[k, :, c0:c0 + cw],
                                in_=stp[:, :cw])

                state = front(0, xTb0)
                for b in range(BL):
                    nxt = front(b + 1) if b + 1 < BL else None
                    if b == 1:
                        prep_fcw(fc1_w.ap(), fc1w_dram, KD, HID)
                    if b == 2:
                        prep_fcw(fc2_w.ap(), fc2w_dram, KH, D)
                    attention(b, *state)
                    state = nxt

            # ============ PHASE B (MLP over global tokens) ============
            with tc.tile_pool(name="pwB", bufs=1) as pwB, \
                 tc.tile_pool(name="pb", bufs=2) as pb, \
                 tc.tile_pool(name="pbh", bufs=1) as pbh, \
                 tc.tile_pool(name="stgB", bufs=2) as stgB, \
                 tc.tile_pool(name="ps_mmB", bufs=3, space="PSUM") as ps_mmB, \
                 tc.tile_pool(name="ps_trB", bufs=2, space="PSUM") as ps_trB:

                def load_bf16(dram, ktiles, ncols, dst):
                    for c0, cw in _chunks(ncols, 512):
                        for k in range(ktiles):
                            nc.sync.dma_start(
                                out=dst[:, k, c0:c0 + cw],
                                in_=dram.ap()[k, :, c0:c0 + cw])

                fc1w = pwB.tile([128, KD, HID], BF16, tag="fc1w")
                load_bf16(fc1w_dram, KD, HID, fc1w)
                fc1b_sb = pp.tile([128, 24], F32, tag="fc1b2")
                fold_bias(ln2b_c, fc1w, HID, fc1_b.ap(), fb2_dram,
                          fc1b_sb, ps_mmB, pb)
                fc2w = pwB.tile([128, KH, D], BF16, tag="fc2w")
                load_bf16(fc2w_dram, KH, D, fc2w)

                for g0, cnt in CH_G:
                    x2c = pb.tile([128, KD, 512], BF16, tag="x2c")
                    for k in range(KD):
                        nc.gpsimd.dma_start(out=x2c[:, k, :cnt],
                                            in_=x2_dram.ap()[k, :, g0:g0 + cnt])
                    mu_b, rs_b = ln_bcast(mu2, rstd2, g0, cnt, ps_mmB, pb)
                    zT = pb.tile([128, KD, 512], BF16, tag="zT")
                    ln_apply(x2c[:, :, :cnt], mu_b, rs_b, zT[:, :, :cnt],
                             cnt, pb)
                    for k in range(KD):
                        nc.vector.tensor_scalar_mul(
                            out=zT[:, k, :cnt], in0=zT[:, k, :cnt],
                            scalar1=ln2w_sb[:, k:k + 1])

                    h_t = pbh.tile([128, KH, 512], BF16, tag="h")
                    for n in range(KH):
                        ps = ps_mmB.tile([128, 512], F32, tag="mmB")
                        for k in range(KD):
                            nc.tensor.matmul(ps[:, :cnt],
                                             fc1w[:, k, n * 128:(n + 1) * 128],
                                             zT[:, k, :cnt],
                                             start=(k == 0), stop=(k == KD - 1))
                        nc.scalar.activation(out=h_t[:, n, :cnt], in_=ps[:, :cnt],
                                             func=AF.Gelu,
                                             bias=fc1b_sb[:, n:n + 1])
                    xf = pb.tile([128, KD, 512], BF16, tag="xf")
                    for n in range(KD):
                        ps = ps_mmB.tile([128, 512], F32, tag="mmB")
                        for k in range(KH):
                            nc.tensor.matmul(ps[:, :cnt],
                                             fc2w[:, k, n * 128:(n + 1) * 128],
                                             h_t[:, k, :cnt],
                                             start=(k == 0), stop=(k == KH - 1))
                        nc.vector.scalar_tensor_tensor(
                            out=xf[:, n, :cnt], in0=ps[:, :cnt],
                            scalar=fc2b_sb[:, n:n + 1],
                            in1=x2c[:, n, :cnt],
                            op0=ALU.add, op1=ALU.add)
                    # ---- transpose back to token-major and store ----
                    for c0, ccnt in _chunks(cnt, 128):
                        om = stgB.tile([128, D], BF16, tag="om")
                        for k in range(KD):
                            ps_tr = ps_trB.tile([128, 128], BF16, tag="tr")
                            nc.tensor.transpose(ps_tr[:ccnt, :],
                                                xf[:, k, c0:c0 + ccnt], ident)
                            if k % 2 == 0:
                                nc.scalar.copy(out=om[:ccnt, k * 128:(k + 1) * 128],
                                               in_=ps_tr[:ccnt, :])
                            else:
                                nc.vector.tensor_copy(
                                    out=om[:ccnt, k * 128:(k + 1) * 128],
                                    in_=ps_tr[:ccnt, :])
                        nc.gpsimd.dma_start(
                            out=out_flat[g0 + c0:g0 + c0 + ccnt, :],
                            in_=om[:ccnt, :])

    nc.compile()
    return nc


def kernel(**inputs) -> np.ndarray:
    if "nc" in _NC_CACHE:
        nc = _NC_CACHE["nc"]
    else:
        nc = _NC_CACHE["nc"] = build_nc()
    x = np.ascontiguousarray(np.asarray(inputs["x"], dtype=np.float32))
    weights = {k: np.ascontiguousarray(np.asarray(v, dtype=np.float32))
               for k, v in inputs.items() if k != "x"}
    in_maps = []
    for c in range(N_CORES):
        m = {"x": x[c * BL:(c + 1) * BL]}
        m.update(weights)
        in_maps.append(m)
    last_err = None
    for attempt in range(3):
        try:
            r = run_bass_kernel_spmd(nc, in_maps, core_ids=list(range(N_CORES)))
            return np.concatenate([r.results[c]["out"] for c in range(N_CORES)],
                                  axis=0)
        except Exception as e:  # transient device flakes: retry
            last_err = e
    raise last_err
